# revision 1
# baseline (speedup 1.0000x reference)
"""Trainium2 Bass kernel for nn_EndpointRegressor (2x TransformerConv GNN +
AttentionalAggregation) distributed over 8 NeuronCores.

Sharding: edges partitioned by destination node range (6272 nodes/core);
each core owns its dst nodes exclusively, so segment softmax/scatter stats
need no cross-core reduction.  Per layer each core computes k|v for its own
nodes, the k|v table is AllGather-ed, and each core gathers k|v rows for its
edge shard with dma_gather.  The segment softmax uses exp without max
subtraction (alpha range is ~±0.09 for this model family) and folds the
denominator division to the node side: out = (sum ex*v)/(sum ex).
Scatter-adds are one-hot matmuls accumulated in PSUM per 128-node window.
"""
import contextlib
import math
import numpy as np

def _nullcm():
    return contextlib.nullcontext()

import concourse.bass as bass
import concourse.bacc as bacc
import concourse.mybir as mybir
import concourse.tile as tile
from concourse._compat import get_trn_type
from concourse.bass_utils import run_bass_kernel_spmd
from concourse.library_config import mlp

# ---- problem constants (fixed by the problem spec) ----
N, E, G = 50000, 500000, 32
H, D = 4, 40
HID = H * D            # 160
JK = 2 * HID           # 320
NCORES = 8
NSHARD = 6272          # 49*128 nodes per core
NPAD = NCORES * NSHARD # 50176
WIN = NSHARD // 128    # 49
SPLIT = NPAD // 2      # 25088: low/high kv-table split (int16 gather indices)
GS = 1024              # slots per dma_gather instruction
INVSQD = 1.0 / math.sqrt(float(D))

f32 = mybir.dt.float32
i16 = mybir.dt.int16


def _wrap16(ix):
    """[n] int16 -> [128, n//16] dma_gather index layout (16-wrap, x8 replicate)."""
    return np.tile(ix.reshape(-1, 16).T, (8, 1))


def _preprocess(x, edge_index, edge_attr, batch):
    """Sort edges by dst, shard by dst range, split each window's edges into
    low/high src groups, pad to uniform chunk counts. Returns per-core input
    arrays + the adaptive chunk capacities (C_L, C_H)."""
    src = np.asarray(edge_index[0], dtype=np.int64)
    dst = np.asarray(edge_index[1], dtype=np.int64)
    ea = np.asarray(edge_attr, dtype=np.float32)
    order = np.argsort(dst, kind="stable")
    src, dst, ea = src[order], dst[order], ea[order]

    core = dst // NSHARD
    win = (dst % NSHARD) // 128
    low = src < SPLIT

    # bucket edge indices per (core, window, low/high)
    buckets = {}
    for r in range(NCORES):
        m_r = core == r
        for w in range(WIN):
            m = m_r & (win == w)
            idx = np.nonzero(m)[0]
            lo = idx[low[idx]]
            hi = idx[~low[idx]]
            buckets[(r, w)] = (lo, hi)

    C_L = max(1, max((len(b[0]) + 127) // 128 for b in buckets.values()))
    C_H = max(1, max((len(b[1]) + 127) // 128 for b in buckets.values()))
    NCH = C_L + C_H
    NGL = (WIN * C_L * 128 + GS - 1) // GS
    NGH = (WIN * C_H * 128 + GS - 1) // GS

    per_core = []
    for r in range(NCORES):
        Lslots = np.zeros(NGL * GS, np.int64)      # gather idx (low table)
        Lvalid = np.zeros(NGL * GS, bool)
        Hslots = np.zeros(NGH * GS, np.int64)
        Hvalid = np.zeros(NGH * GS, bool)
        eaT = np.zeros((WIN, 5, NCH * 128), np.float32)
        eaT[:, 4, :] = 1.0                         # bias row for the e-matmul
        dstrel = np.full((WIN, 128, NCH), -1.0, np.float32)
        for w in range(WIN):
            lo, hi = buckets[(r, w)]
            for (idx_e, slots, valid, base_c, j0, table_off) in (
                (lo, Lslots, Lvalid, w * C_L, 0, 0),
                (hi, Hslots, Hvalid, w * C_H, C_L, SPLIT),
            ):
                n = len(idx_e)
                s0 = base_c * 128
                slots[s0 : s0 + n] = src[idx_e] - table_off
                valid[s0 : s0 + n] = True
                # pipeline slot (w, j0 + k//128, k%128)
                kk = np.arange(n)
                jj = j0 + kk // 128
                pp = kk % 128
                eaT[w, :4, :][:, jj * 128 + pp] = ea[idx_e].T
                dstrel[w, pp, jj] = (dst[idx_e] % 128).astype(np.float32)
        # pad slots keep idx=0: every gather slot must be WRITTEN on hw
        # (unwritten SBUF can hold NaN garbage that poisons 0*NaN in the
        # scatter matmul), so no -1 skip sentinels.
        # own-node arrays
        n0 = r * NSHARD
        xT = np.zeros((5, NSHARD), np.float32)
        batchc = np.full((WIN, 128, 1), -1.0, np.float32)
        n_real = max(0, min(NSHARD, N - n0))
        if n_real > 0:
            xT[:, :n_real] = np.asarray(x[n0 : n0 + n_real], np.float32).T
            bc = np.asarray(batch[n0 : n0 + n_real], np.float32).reshape(-1, 1)
            batchc.reshape(NSHARD, 1)[:n_real] = bc
        per_core.append(
            dict(
                xT=xT,
                idxL=np.ascontiguousarray(_wrap16(Lslots.astype(np.int16))),
                idxH=np.ascontiguousarray(_wrap16(Hslots.astype(np.int16))),
                eaT=eaT,
                dstrel=dstrel,
                batchc=batchc,
            )
        )
    return per_core, C_L, C_H


def _weights(inp):
    """Host-side weight packing (bias folding, concat layouts)."""
    w = {}
    b_in = inp["b_in"].astype(np.float64)
    w["iota128"] = np.broadcast_to(np.arange(128, dtype=np.float32), (128, 128)).copy()
    w["iota32"] = np.broadcast_to(np.arange(32, dtype=np.float32), (128, 32)).copy()
    w["ident"] = np.eye(128, dtype=np.float32)
    Wg1 = inp["Wg1"].astype(np.float32)
    w["wg1_h1"] = np.concatenate([Wg1[:HID], inp["bg1"].astype(np.float32)[None, :]], 0)   # [161,160]
    w["wg1_h2"] = np.concatenate([Wg1[HID:], np.zeros((1, HID), np.float32)], 0)           # [161,160]
    w["wg2rep"] = np.broadcast_to(inp["Wg2"].astype(np.float32)[:, 0], (128, HID)).copy()
    w["wh1"] = np.concatenate([inp["Wh1"].astype(np.float32), inp["bh1"].astype(np.float32)[None, :]], 0)  # [321,320]
    w["wh2"] = np.concatenate([inp["Wh2"].astype(np.float32), inp["bh2"].astype(np.float32)[None, :]], 0)  # [321,6]
    w["win"] = inp["W_in"].astype(np.float32)  # [5,160]
    w["bg2rep"] = np.full((128, 1), float(np.asarray(inp["bg2"]).reshape(-1)[0]), np.float32)
    for layer in range(2):
        Wq, Wk, Wv = (inp[k][layer].astype(np.float64) for k in ("Wq", "Wk", "Wv"))
        bq, bk, bv = (inp[k][layer].astype(np.float64) for k in ("bq", "bk", "bv"))
        Wskip, bskip = inp["Wskip"][layer].astype(np.float64), inp["bskip"][layer].astype(np.float64)
        Wbeta = inp["Wbeta"][layer].astype(np.float64)
        We, be = inp["We"][layer].astype(np.float64), inp["be"][layer].astype(np.float64)
        if layer == 0:
            bq, bk, bv, bskip = bq + b_in @ Wq, bk + b_in @ Wk, bv + b_in @ Wv, bskip + b_in @ Wskip
        P = (Wbeta[:HID, 0] + Wbeta[2 * HID :, 0])
        Q = (Wbeta[HID : 2 * HID, 0] - Wbeta[2 * HID :, 0])
        w[f"wkv{layer}"] = np.concatenate(
            [np.concatenate([Wk, Wv], 1), np.concatenate([bk, bv])[None, :]], 0
        ).astype(np.float32)                                           # [161,320]
        w[f"wq{layer}"] = np.concatenate([Wq, bq[None, :]], 0).astype(np.float32)  # [161,160]
        rb = np.concatenate([Wskip, (Wskip @ Q)[:, None]], 1)          # [160,161]
        rb_b = np.concatenate([bskip, [bskip @ Q]])[None, :]           # [1,161]
        w[f"wrb{layer}"] = np.concatenate([rb, rb_b], 0).astype(np.float32)        # [161,161]
        w[f"prep{layer}"] = np.broadcast_to(P.astype(np.float32), (128, HID)).copy()
        w[f"wekv{layer}"] = np.concatenate(
            [np.concatenate([We, We], 1), np.concatenate([be, be])[None, :]], 0
        ).astype(np.float32)                                           # [5,320]
    return w


def _build(C_L, C_H, phases="full", winlim=None, skips=()):
    skips = set(skips)
    NCH = C_L + C_H
    NGL = (WIN * C_L * 128 + GS - 1) // GS
    NGH = (WIN * C_H * 128 + GS - 1) // GS

    nc = bacc.Bacc(get_trn_type() or "TRN2", target_bir_lowering=False)

    # ---- dram I/O ----
    d = {}
    d["xT"] = nc.dram_tensor("xT", [5, NSHARD], f32, kind="ExternalInput")
    d["idxL"] = nc.dram_tensor("idxL", [128, NGL * GS // 16], i16, kind="ExternalInput")
    d["idxH"] = nc.dram_tensor("idxH", [128, NGH * GS // 16], i16, kind="ExternalInput")
    d["eaT"] = nc.dram_tensor("eaT", [WIN, 5, NCH * 128], f32, kind="ExternalInput")
    d["dstrel"] = nc.dram_tensor("dstrel", [WIN, 128, NCH], f32, kind="ExternalInput")
    d["batchc"] = nc.dram_tensor("batchc", [WIN, 128, 1], f32, kind="ExternalInput")
    wshapes = dict(
        iota128=[128, 128], iota32=[128, 32], ident=[128, 128],
        wg1_h1=[161, HID], wg1_h2=[161, HID], wg2rep=[128, HID],
        wh1=[321, JK], wh2=[321, 6], win=[5, HID], bg2rep=[128, 1],
    )
    for layer in range(2):
        wshapes[f"wkv{layer}"] = [161, JK]
        wshapes[f"wq{layer}"] = [161, HID]
        wshapes[f"wrb{layer}"] = [161, 161]
        wshapes[f"prep{layer}"] = [128, HID]
        wshapes[f"wekv{layer}"] = [5, JK]
    for k, shp in wshapes.items():
        d[k] = nc.dram_tensor(k, shp, f32, kind="ExternalInput")
    out_d = nc.dram_tensor("out", [32, 6], f32, kind="ExternalOutput")
    dbg_d = nc.dram_tensor("dbg", [128, JK], f32, kind="ExternalOutput")

    hT = [nc.dram_tensor(f"hT{i}", [HID, NSHARD], f32) for i in range(3)]
    h_nm = [None, nc.dram_tensor("h_nm1", [NSHARD, HID], f32),
            nc.dram_tensor("h_nm2", [NSHARD, HID], f32)]
    kv_own = [nc.dram_tensor(f"kv_own{l}", [NSHARD, JK], f32) for l in range(2)]
    kv_full = [nc.dram_tensor(f"kv_full{l}", [NPAD, JK], f32, addr_space="Shared")
               for l in range(2)]
    pool_in = nc.dram_tensor("pool_in", [32, JK + 1], f32)
    pool_out = nc.dram_tensor("pool_out", [32, JK + 1], f32, addr_space="Shared")
    rg = [list(range(NCORES))]

    with tile.TileContext(nc) as tc:
        with (
            tc.tile_pool(name="const", bufs=1) as cst,
            tc.tile_pool(name="sb", bufs=2) as sb,
            tc.tile_pool(name="gath", bufs=3) as gath,
            tc.tile_pool(name="ps", bufs=2, space="PSUM") as ps,
        ):
            nc.gpsimd.load_library(mlp)
            regGS = nc.gpsimd.to_reg(GS)

            # ---- persistent constants ----
            C = {}
            def _load_const(key, part, cols, row0=0):
                t = cst.tile([part, cols], f32, name=f"c_{key}_{row0}")
                nc.sync.dma_start(out=t[:], in_=d[key][row0 : row0 + part, :])
                return t
            for layer in range(2):
                C[f"wkv{layer}a"] = _load_const(f"wkv{layer}", 128, JK)
                C[f"wkv{layer}b"] = _load_const(f"wkv{layer}", 32, JK, 128)
                C[f"wkv{layer}c"] = _load_const(f"wkv{layer}", 1, JK, 160)
                C[f"wq{layer}a"] = _load_const(f"wq{layer}", 128, HID)
                C[f"wq{layer}b"] = _load_const(f"wq{layer}", 32, HID, 128)
                C[f"wq{layer}c"] = _load_const(f"wq{layer}", 1, HID, 160)
                C[f"wrb{layer}a"] = _load_const(f"wrb{layer}", 128, 161)
                C[f"wrb{layer}b"] = _load_const(f"wrb{layer}", 32, 161, 128)
                C[f"wrb{layer}c"] = _load_const(f"wrb{layer}", 1, 161, 160)
                C[f"prep{layer}"] = _load_const(f"prep{layer}", 128, HID)
                C[f"wekv{layer}"] = _load_const(f"wekv{layer}", 5, JK)
            C["iota128"] = _load_const("iota128", 128, 128)
            C["iota32"] = _load_const("iota32", 128, 32)
            C["ident"] = _load_const("ident", 128, 128)
            C["wg2rep"] = _load_const("wg2rep", 128, HID)
            for key in ("wg1_h1", "wg1_h2"):
                C[key + "a"] = _load_const(key, 128, HID)
                C[key + "b"] = _load_const(key, 32, HID, 128)
                C[key + "c"] = _load_const(key, 1, HID, 160)
            C["bg2rep"] = _load_const("bg2rep", 128, 1)
            C["wh1a"] = _load_const("wh1", 128, JK)
            C["wh1b"] = _load_const("wh1", 128, JK, 128)
            C["wh1c"] = _load_const("wh1", 64, JK, 256)
            C["wh1d"] = _load_const("wh1", 1, JK, 320)
            C["wh2a"] = _load_const("wh2", 128, 6)
            C["wh2b"] = _load_const("wh2", 128, 6, 128)
            C["wh2c"] = _load_const("wh2", 64, 6, 256)
            C["wh2d"] = _load_const("wh2", 1, 6, 320)
            C["win"] = _load_const("win", 5, HID)

            idxLt = cst.tile([128, NGL * GS // 16], i16, name="idxLt")
            nc.sync.dma_start(out=idxLt[:], in_=d["idxL"][:])
            idxHt = cst.tile([128, NGH * GS // 16], i16, name="idxHt")
            nc.sync.dma_start(out=idxHt[:], in_=d["idxH"][:])

            # ---- phase 0: h0T = (x @ W_in)^T, own nodes ----
            with nc.named_scope("p0"):
                NT0 = (NSHARD + 511) // 512
                for t in range(NT0):
                    c0, cn = t * 512, min(512, NSHARD - t * 512)
                    xts = sb.tile([5, cn], f32, tag="xts")
                    nc.sync.dma_start(out=xts[:], in_=d["xT"][:, c0 : c0 + cn])
                    for (r0, m) in ((0, 128), (128, 32)):
                        ph = ps.tile([m, cn], f32, tag="kve", bufs=3)
                        nc.tensor.matmul(ph[:], C["win"][:, r0 : r0 + m], xts[:],
                                         start=True, stop=True)
                        hsb = sb.tile([m, cn], f32, tag="hsb")
                        nc.vector.tensor_copy(out=hsb[:], in_=ph[:])
                        nc.sync.dma_start(out=hT[0][r0 : r0 + m, c0 : c0 + cn], in_=hsb[:])

            ones1 = cst.tile([1, 128], f32, name="ones1")
            nc.gpsimd.memset(ones1[:], 1.0)

            # ==== two layers ====
            nlayers = 0 if phases == "p0" else (1 if phases in ("kv", "edge0") else 2)
            for layer in range(nlayers):
                hsrc = hT[layer]
                # ---- kv GEMM own nodes -> kv_own ----
                with nc.named_scope(f"kv{layer}"):
                    for t in range(WIN):
                        csl = slice(t * 128, (t + 1) * 128)
                        hta = sb.tile([128, 128], f32, tag="hta", bufs=3)
                        nc.sync.dma_start(out=hta[:], in_=hsrc[0:128, csl])
                        htb = sb.tile([32, 128], f32, tag="htb", bufs=3)
                        nc.sync.dma_start(out=htb[:], in_=hsrc[128:160, csl])
                        pkv = ps.tile([128, JK], f32, tag="kve", bufs=3)
                        nc.tensor.matmul(pkv[:], hta[:], C[f"wkv{layer}a"][:], start=True, stop=False)
                        nc.tensor.matmul(pkv[:], htb[:], C[f"wkv{layer}b"][:], start=False, stop=False)
                        nc.tensor.matmul(pkv[:], ones1[:, :128], C[f"wkv{layer}c"][:], start=False, stop=True)
                        kvsb = sb.tile([128, JK], f32, tag="kvsb")
                        nc.vector.tensor_copy(out=kvsb[:], in_=pkv[:])
                        nc.sync.dma_start(out=kv_own[layer][csl, :], in_=kvsb[:])
                with nc.named_scope(f"ag{layer}"):
                    nc.gpsimd.collective_compute(
                        "AllGather", mybir.AluOpType.bypass, replica_groups=rg,
                        ins=[kv_own[layer][:]], outs=[kv_full[layer][:]])
                if layer == 0:
                    dbgt = sb.tile([128, JK], f32, tag="dbgt")
                    nc.sync.dma_start(out=dbgt[:], in_=kv_full[0][13000:13128, :])
                    nc.sync.dma_start(out=dbg_d[:], in_=dbgt[:])

                # ---- edge phase ----
                if phases == "kv":
                    break
                with nc.named_scope(f"edge{layer}"):
                    cur = {"L": -1, "H": -1}
                    cur_tile = {"L": None, "H": None}

                    def _gather(region, gt):
                        if cur[region] == gt:
                            return cur_tile[region]
                        idxt, base, ng = (
                            (idxLt, 0, NGL) if region == "L" else (idxHt, SPLIT, NGH)
                        )
                        gtile = gath.tile([128, GS // 128, JK], f32, tag="g" + region)
                        nc.gpsimd.dma_gather(
                            gtile[:],
                            kv_full[layer][base : base + SPLIT, :],
                            idxt[:, gt * (GS // 16) : (gt + 1) * (GS // 16)],
                            num_idxs=GS, num_idxs_reg=regGS, elem_size=JK)
                        cur[region] = gt
                        cur_tile[region] = gtile
                        return gtile

                    for w in range(WIN if winlim is None else winlim):
                        wsl = slice(w * 128, (w + 1) * 128)
                        eaw = sb.tile([5, NCH * 128], f32, tag="eaw", bufs=3)
                        nc.sync.dma_start(out=eaw[:], in_=d["eaT"][w])
                        drw = sb.tile([128, NCH], f32, tag="drw", bufs=3)
                        nc.sync.dma_start(out=drw[:], in_=d["dstrel"][w])
                        hta = sb.tile([128, 128], f32, tag="hta", bufs=3)
                        nc.sync.dma_start(out=hta[:], in_=hsrc[0:128, wsl])
                        htb = sb.tile([32, 128], f32, tag="htb", bufs=3)
                        nc.sync.dma_start(out=htb[:], in_=hsrc[128:160, wsl])
                        # q for this window
                        pq = ps.tile([128, HID], f32, tag="qrb", bufs=1)
                        nc.tensor.matmul(pq[:], hta[:], C[f"wq{layer}a"][:], start=True, stop=False)
                        nc.tensor.matmul(pq[:], htb[:], C[f"wq{layer}b"][:], start=False, stop=False)
                        nc.tensor.matmul(pq[:], ones1[:, :128], C[f"wq{layer}c"][:], start=False, stop=True)
                        qw = sb.tile([128, HID], f32, tag="qw", bufs=3)
                        nc.vector.tensor_copy(out=qw[:], in_=pq[:])
                        # r / beta-partial for this window
                        prb = ps.tile([128, 161], f32, tag="qrb", bufs=1)
                        nc.tensor.matmul(prb[:], hta[:], C[f"wrb{layer}a"][:], start=True, stop=False)
                        nc.tensor.matmul(prb[:], htb[:], C[f"wrb{layer}b"][:], start=False, stop=False)
                        nc.tensor.matmul(prb[:], ones1[:, :128], C[f"wrb{layer}c"][:], start=False, stop=True)
                        rsb = sb.tile([128, 161], f32, tag="rsb", bufs=3)
                        nc.vector.tensor_copy(out=rsb[:], in_=prb[:])

                        pacc = ps.tile([128, 164], f32, tag="acc")
                        for j in range(NCH):
                            if j < C_L:
                                cidx = w * C_L + j
                                gtile = _gather("L", cidx // (GS // 128))
                            else:
                                cidx = w * C_H + (j - C_L)
                                gtile = _gather("H", cidx // (GS // 128))
                            sub = cidx % (GS // 128)
                            kv_g = gtile[:, sub, :]
                            # e = ea @ We (+bias) in PSUM
                            pe = ps.tile([128, JK], f32, tag="kve", bufs=3)
                            nc.tensor.matmul(pe[:], eaw[:, j * 128 : (j + 1) * 128],
                                             C[f"wekv{layer}"][:], start=True, stop=True)
                            # kv_e = kv_g + e
                            kve = sb.tile([128, JK], f32, tag="kvesb", bufs=4)
                            nc.vector.tensor_tensor(out=kve[:], in0=pe[:], in1=kv_g,
                                                    op=mybir.AluOpType.add)
                            # S^T one-hot [edges, nodes]
                            st = sb.tile([128, 128], f32, tag="st", bufs=4)
                            nc.vector.tensor_tensor(
                                out=st[:], in0=drw[:, j : j + 1].to_broadcast([128, 128]),
                                in1=C["iota128"][:], op=mybir.AluOpType.is_equal)
                            # S = (S^T)^T via PE transpose
                            if "qg" not in skips:
                                pst = ps.tile([128, 128], f32, tag="tp")
                                nc.tensor.transpose(pst[:], st[:], C["ident"][:])
                                ssb = sb.tile([128, 128], f32, tag="ssb", bufs=4)
                                nc.vector.tensor_copy(out=ssb[:], in_=pst[:])
                                # q gathered to edges
                                pqg = ps.tile([128, HID], f32, tag="tp")
                                nc.tensor.matmul(pqg[:], ssb[:], qw[:], start=True, stop=True)
                                qsrc = pqg[:]
                            else:
                                qsrc = kve[:, :HID]
                            # alpha = sum_d q_g * k_e per head
                            tq = sb.tile([128, HID], f32, tag="tq", bufs=4)
                            nc.vector.tensor_tensor(out=tq[:], in0=qsrc, in1=kve[:, :HID],
                                                    op=mybir.AluOpType.mult)
                            al = sb.tile([128, H], f32, tag="al", bufs=4)
                            nc.vector.tensor_reduce(
                                out=al[:], in_=tq[:].rearrange("p (h dd) -> p h dd", h=H),
                                axis=mybir.AxisListType.X, op=mybir.AluOpType.add)
                            # w tile: [v_e * ex | ex]
                            wt = sb.tile([128, 164], f32, tag="wt", bufs=4)
                            if "exp" not in skips:
                                nc.scalar.activation(out=wt[:, 160:164], in_=al[:],
                                                     func=mybir.ActivationFunctionType.Exp,
                                                     scale=INVSQD)
                            else:
                                nc.vector.tensor_copy(out=wt[:, 160:164], in_=al[:])
                            nc.vector.tensor_tensor(
                                out=wt[:, :HID].rearrange("p (h dd) -> p h dd", h=H),
                                in0=kve[:, HID:].rearrange("p (h dd) -> p h dd", h=H),
                                in1=wt[:, 160:164].rearrange("p (h o) -> p h o", h=H).to_broadcast([128, H, D]),
                                op=mybir.AluOpType.mult)
                            # scatter: acc[nodes] += S^T.T @ [w | ex]
                            nc.tensor.matmul(pacc[:], st[:], wt[:],
                                             start=(j == 0), stop=(j == NCH - 1),
                                             skip_group_check=True)

                        # ---- window post: out = num/den, beta gate, h' ----
                        accsb = sb.tile([128, 164], f32, tag="accsb")
                        nc.vector.tensor_copy(out=accsb[:], in_=pacc[:])
                        dmax = sb.tile([128, H], f32, tag="dmax")
                        nc.vector.tensor_scalar_max(dmax[:], accsb[:, 160:164], 1e-30)
                        denr = sb.tile([128, H], f32, tag="denr")
                        nc.vector.reciprocal(out=denr[:], in_=dmax[:])
                        outn = sb.tile([128, HID], f32, tag="outn")
                        nc.vector.tensor_tensor(
                            out=outn[:].rearrange("p (h dd) -> p h dd", h=H),
                            in0=accsb[:, :HID].rearrange("p (h dd) -> p h dd", h=H),
                            in1=denr[:].rearrange("p (h o) -> p h o", h=H).to_broadcast([128, H, D]),
                            op=mybir.AluOpType.mult)
                        scr = sb.tile([128, HID], f32, tag="scr")
                        outP = sb.tile([128, 1], f32, tag="outP")
                        nc.vector.tensor_tensor(out=scr[:], in0=outn[:],
                            in1=C[f"prep{layer}"][:], op=mybir.AluOpType.mult)
                        nc.vector.tensor_reduce(out=outP[:],
                            in_=scr[:].rearrange("p (a b) -> p a b", a=1),
                            axis=mybir.AxisListType.XY, op=mybir.AluOpType.add)
                        beta = sb.tile([128, 1], f32, tag="beta")
                        if "sig" not in skips:
                            nc.scalar.activation(out=beta[:], in_=outP[:],
                                                 func=mybir.ActivationFunctionType.Sigmoid,
                                                 bias=rsb[:, 160:161], scale=1.0)
                        else:
                            nc.vector.tensor_copy(out=beta[:], in_=outP[:])
                        dvec = sb.tile([128, HID], f32, tag="dvec")
                        nc.vector.tensor_sub(dvec[:], rsb[:, :HID], outn[:])
                        hp = sb.tile([128, HID], f32, tag="hp")
                        if "stt" not in skips:
                            nc.vector.scalar_tensor_tensor(
                                out=hp[:], in0=dvec[:], scalar=beta[:, 0:1], in1=outn[:],
                                op0=mybir.AluOpType.mult, op1=mybir.AluOpType.add)
                        else:
                            nc.vector.tensor_scalar_mul(hp[:], dvec[:], beta[:, 0:1])
                            nc.vector.tensor_add(hp[:], hp[:], outn[:])
                        nc.sync.dma_start(out=h_nm[layer + 1][wsl, :], in_=hp[:])
                        # transpose h' into hT[layer+1]
                        if "trans" in skips:
                            continue
                        ptr1 = ps.tile([128, 128], f32, tag="tp")
                        nc.tensor.transpose(ptr1[:], hp[:, 0:128], C["ident"][:])
                        t1 = sb.tile([128, 128], f32, tag="t1")
                        nc.vector.tensor_copy(out=t1[:], in_=ptr1[:])
                        nc.sync.dma_start(out=hT[layer + 1][0:128, wsl], in_=t1[:])
                        ptr2 = ps.tile([32, 128], f32, tag="tp")
                        nc.tensor.transpose(ptr2[:], hp[:, 128:160], C["ident"][:])
                        t2 = sb.tile([32, 128], f32, tag="t2")
                        nc.vector.tensor_copy(out=t2[:], in_=ptr2[:])
                        nc.sync.dma_start(out=hT[layer + 1][128:160, wsl], in_=t2[:])

            if phases == "p0":
                dbgt = sb.tile([128, JK], f32, tag="dbgt")
                nc.gpsimd.memset(dbgt[:], 0.0)
                nc.sync.dma_start(out=dbgt[:, :160], in_=hT[0][0:128, 999:1159])
                nc.sync.dma_start(out=dbg_d[:], in_=dbgt[:])
            # ==== final phase: gate + graph pooling + head MLP ====
            if phases != "full":
                dummy = sb.tile([32, 6], f32, tag="osb")
                nc.gpsimd.memset(dummy[:], 0.0)
                nc.sync.dma_start(out=out_d[:], in_=dummy[:])
            if phases == "full":
              with nc.named_scope("final"):
                pgr = ps.tile([32, JK + 1], f32, tag="acc")
                for w in range(WIN):
                    wsl = slice(w * 128, (w + 1) * 128)
                    h1w = sb.tile([128, HID], f32, tag="h1w")
                    nc.sync.dma_start(out=h1w[:], in_=h_nm[1][wsl, :])
                    h2w = sb.tile([128, HID], f32, tag="h2w")
                    nc.sync.dma_start(out=h2w[:], in_=h_nm[2][wsl, :])
                    bcw = sb.tile([128, 1], f32, tag="bcw")
                    nc.sync.dma_start(out=bcw[:], in_=d["batchc"][w])
                    pg = ps.tile([128, HID], f32, tag="kve", bufs=3)
                    first = True
                    for (src_hT, wkey) in ((hT[1], "wg1_h1"), (hT[2], "wg1_h2")):
                        g_a = sb.tile([128, 128], f32, tag="hta", bufs=3)
                        nc.sync.dma_start(out=g_a[:], in_=src_hT[0:128, wsl])
                        g_b = sb.tile([32, 128], f32, tag="htb", bufs=3)
                        nc.sync.dma_start(out=g_b[:], in_=src_hT[128:160, wsl])
                        nc.tensor.matmul(pg[:], g_a[:], C[wkey + "a"][:], start=first, stop=False)
                        first = False
                        nc.tensor.matmul(pg[:], g_b[:], C[wkey + "b"][:], start=False, stop=False)
                    nc.tensor.matmul(pg[:], ones1[:, :128], C["wg1_h1c"][:], start=False, stop=True)
                    grelu = sb.tile([128, HID], f32, tag="grelu")
                    nc.scalar.activation(out=grelu[:], in_=pg[:],
                                         func=mybir.ActivationFunctionType.Relu)
                    scr2 = sb.tile([128, HID], f32, tag="scr")
                    gatec = sb.tile([128, 1], f32, tag="gatec")
                    nc.vector.tensor_tensor(out=scr2[:], in0=grelu[:],
                        in1=C["wg2rep"][:], op=mybir.AluOpType.mult)
                    nc.vector.tensor_reduce(out=gatec[:],
                        in_=scr2[:].rearrange("p (a b) -> p a b", a=1),
                        axis=mybir.AxisListType.XY, op=mybir.AluOpType.add)
                    ge = sb.tile([128, 1], f32, tag="ge")
                    nc.scalar.activation(out=ge[:], in_=gatec[:],
                                         func=mybir.ActivationFunctionType.Exp,
                                         bias=C["bg2rep"][:, 0:1])
                    sg = sb.tile([128, 32], f32, tag="sg")
                    nc.vector.tensor_tensor(out=sg[:], in0=bcw[:].to_broadcast([128, 32]),
                                            in1=C["iota32"][:], op=mybir.AluOpType.is_equal)
                    wg = sb.tile([128, JK + 1], f32, tag="wg")
                    nc.vector.tensor_scalar_mul(wg[:, 0:HID], h1w[:], ge[:, 0:1])
                    nc.vector.tensor_scalar_mul(wg[:, HID:JK], h2w[:], ge[:, 0:1])
                    nc.vector.tensor_copy(out=wg[:, JK : JK + 1], in_=ge[:])
                    nc.tensor.matmul(pgr[:], sg[:], wg[:], start=(w == 0),
                                     stop=(w == WIN - 1), skip_group_check=True)
                pg_sb = sb.tile([32, JK + 1], f32, tag="pg_sb")
                nc.vector.tensor_copy(out=pg_sb[:], in_=pgr[:])
                nc.sync.dma_start(out=pool_in[:], in_=pg_sb[:])
                nc.gpsimd.collective_compute(
                    "AllReduce", mybir.AluOpType.add, replica_groups=rg,
                    ins=[pool_in[:]], outs=[pool_out[:]])
                psb = sb.tile([32, JK + 1], f32, tag="psb")
                nc.sync.dma_start(out=psb[:], in_=pool_out[:])
                gden = sb.tile([32, 1], f32, tag="gden")
                nc.vector.tensor_scalar_max(gden[:], psb[:, JK : JK + 1], 1e-30)
                gdr = sb.tile([32, 1], f32, tag="gdr")
                nc.vector.reciprocal(out=gdr[:], in_=gden[:])
                pl = sb.tile([32, JK], f32, tag="pl")
                nc.vector.tensor_scalar_mul(pl[:], psb[:, 0:JK], gdr[:, 0:1])

                def _headmm(vin, wa, wb, wc, wd, nout, tagp):
                    """vin [32, 320] @ W[320, nout] + bias via PE transposes."""
                    pouts = ps.tile([32, nout], f32, tag=tagp, bufs=(3 if tagp == "kve" else 1))
                    for si, (c0, m) in enumerate(((0, 128), (128, 128), (256, 64))):
                        ptt = ps.tile([m, 32], f32, tag="tp")
                        nc.tensor.transpose(ptt[:], vin[:, c0 : c0 + m], C["ident"][0:32, 0:32])
                        tsb = sb.tile([m, 32], f32, tag="tsb")
                        nc.vector.tensor_copy(out=tsb[:], in_=ptt[:])
                        nc.tensor.matmul(pouts[:], tsb[:], (wa, wb, wc)[si][:m, :],
                                         start=(si == 0), stop=False, skip_group_check=True)
                    nc.tensor.matmul(pouts[:], ones1[:, :32], wd[:],
                                     start=False, stop=True, skip_group_check=True)
                    return pouts

                ph1 = _headmm(pl, C["wh1a"], C["wh1b"], C["wh1c"], C["wh1d"], JK, "qrb")
                vrel = sb.tile([32, JK], f32, tag="vrel")
                nc.scalar.activation(out=vrel[:], in_=ph1[:],
                                     func=mybir.ActivationFunctionType.Relu)
                ph2 = _headmm(vrel, C["wh2a"], C["wh2b"], C["wh2c"], C["wh2d"], 6, "kve")
                osb = sb.tile([32, 6], f32, tag="osb")
                nc.vector.tensor_copy(out=osb[:], in_=ph2[:])
                nc.sync.dma_start(out=out_d[:], in_=osb[:])

    nc.compile()
    return nc


_CACHE = {}
_LAST_RES = None


def kernel(**inputs):
    inputs = {k: np.asarray(v) for k, v in inputs.items()}
    per_core, C_L, C_H = _preprocess(
        inputs["x"], inputs["edge_index"], inputs["edge_attr"], inputs["batch"])
    w = _weights(inputs)
    import os as _os
    phases = _os.environ.get("KERNEL_PHASES", "full")
    winlim = _os.environ.get("KERNEL_WINLIM")
    winlim = int(winlim) if winlim else None
    skips = tuple(s for s in _os.environ.get("KERNEL_SKIP", "").split(",") if s)
    key = (C_L, C_H, phases, winlim, skips)
    if key not in _CACHE:
        _CACHE[key] = _build(C_L, C_H, phases, winlim, skips)
    nc = _CACHE[key]
    in_maps = []
    for r in range(NCORES):
        m = dict(w)
        m.update(per_core[r])
        in_maps.append(m)
    import os
    trace = bool(os.environ.get("KERNEL_TRACE"))
    if trace:
        try:
            import axon_prof
            axon_prof.install()
        except Exception:
            trace = False
    res = run_bass_kernel_spmd(nc, in_maps, core_ids=list(range(NCORES)), trace=trace)
    if trace and res.exec_time_ns is not None:
        print(f"HW exec time: {res.exec_time_ns} ns")
        if res.per_core_scope_times:
            for scope, cores in sorted(res.per_core_scope_times.items()):
                print(f"  scope {scope}: {cores}")
    global _LAST_RES
    _LAST_RES = res
    out = res.results[0]["out"]
    return out.reshape(G, 2, 3).astype(np.float32)



# revision 6
# speedup vs baseline: 1.7972x; 1.7972x over previous
"""Trainium2 Bass kernel for nn_EndpointRegressor (2x TransformerConv GNN +
AttentionalAggregation) distributed over 8 NeuronCores.

Sharding: edges partitioned by destination node range (6272 nodes/core);
each core owns its dst nodes exclusively, so segment softmax/scatter stats
need no cross-core reduction.  Per layer each core computes k|v for its own
nodes, the k|v table is AllGather-ed, and each core gathers k|v rows for its
edge shard with dma_gather.  The segment softmax uses exp without max
subtraction (alpha range is ~±0.09 for this model family) and folds the
denominator division to the node side: out = (sum ex*v)/(sum ex).
Scatter-adds are one-hot matmuls accumulated in PSUM per 128-node window.
All matmul operands are bf16 (PE runs 1 cycle/row vs 4 for fp32); PSUM
accumulation stays fp32.  kv table rows padded to 384 bf16 elems (768B) to
satisfy dma_gather's 256B-multiple row constraint.
"""
import math
import numpy as np
import ml_dtypes

import concourse.bass as bass
import concourse.bacc as bacc
import concourse.mybir as mybir
import concourse.tile as tile
from concourse._compat import get_trn_type
from concourse.bass_utils import run_bass_kernel_spmd
from concourse.library_config import mlp

# ---- problem constants (fixed by the problem spec) ----
N, E, G = 50000, 500000, 32
H, D = 4, 40
HID = H * D            # 160
JK = 2 * HID           # 320
KVP = 384              # kv row padded to 384 bf16 elems = 768B (256B multiple)
NCORES = 8
NSHARD = 6272          # 49*128 nodes per core
NPAD = NCORES * NSHARD # 50176
WIN = NSHARD // 128    # 49
SPLIT = NPAD // 2      # 25088: low/high kv-table split (int16 gather indices)
GS = 1024              # slots per dma_gather instruction
INVSQD = 1.0 / math.sqrt(float(D))

f32 = mybir.dt.float32
bf16 = mybir.dt.bfloat16
i16 = mybir.dt.int16
nbf = ml_dtypes.bfloat16


def _wrap16(ix):
    """[n] int16 -> [128, n//16] dma_gather index layout (16-wrap, x8 replicate)."""
    return np.tile(ix.reshape(-1, 16).T, (8, 1))


def _preprocess(x, edge_index, edge_attr, batch):
    """Sort edges by dst, shard by dst range, split each window's edges into
    low/high src groups, pad to uniform chunk counts. Returns per-core input
    arrays + the adaptive chunk capacities (C_L, C_H)."""
    src = np.asarray(edge_index[0], dtype=np.int64)
    dst = np.asarray(edge_index[1], dtype=np.int64)
    ea = np.asarray(edge_attr, dtype=np.float32)
    order = np.argsort(dst, kind="stable")
    src, dst, ea = src[order], dst[order], ea[order]

    core = dst // NSHARD
    win = (dst % NSHARD) // 128
    low = src < SPLIT

    # bucket edge indices per (core, window, low/high)
    buckets = {}
    for r in range(NCORES):
        m_r = core == r
        for w in range(WIN):
            m = m_r & (win == w)
            idx = np.nonzero(m)[0]
            lo = idx[low[idx]]
            hi = idx[~low[idx]]
            buckets[(r, w)] = (lo, hi)

    C_L = max(1, max((len(b[0]) + 127) // 128 for b in buckets.values()))
    C_H = max(1, max((len(b[1]) + 127) // 128 for b in buckets.values()))
    NCH = C_L + C_H
    NGL = (WIN * C_L * 128 + GS - 1) // GS
    NGH = (WIN * C_H * 128 + GS - 1) // GS

    per_core = []
    for r in range(NCORES):
        Lslots = np.zeros(NGL * GS, np.int64)      # gather idx (low table)
        Hslots = np.zeros(NGH * GS, np.int64)
        eaT = np.zeros((WIN, 5, NCH * 128), np.float32)
        eaT[:, 4, :] = 1.0                         # bias row for the e-matmul
        dstrel = np.full((WIN, 128, NCH), -1.0, np.float32)
        for w in range(WIN):
            lo, hi = buckets[(r, w)]
            for (idx_e, slots, base_c, j0, table_off) in (
                (lo, Lslots, w * C_L, 0, 0),
                (hi, Hslots, w * C_H, C_L, SPLIT),
            ):
                n = len(idx_e)
                s0 = base_c * 128
                slots[s0 : s0 + n] = src[idx_e] - table_off
                # pipeline slot (w, j0 + k//128, k%128)
                kk = np.arange(n)
                jj = j0 + kk // 128
                pp = kk % 128
                eaT[w, :4, :][:, jj * 128 + pp] = ea[idx_e].T
                dstrel[w, pp, jj] = (dst[idx_e] % 128).astype(np.float32)
        # pad slots keep idx=0: every gather slot must be WRITTEN on hw
        # (unwritten SBUF can hold NaN garbage that poisons 0*NaN in the
        # scatter matmul), so no -1 skip sentinels.
        # own-node arrays
        n0 = r * NSHARD
        xT = np.zeros((5, NSHARD), np.float32)
        batchc = np.full((WIN, 128, 1), -1.0, np.float32)
        n_real = max(0, min(NSHARD, N - n0))
        if n_real > 0:
            xT[:, :n_real] = np.asarray(x[n0 : n0 + n_real], np.float32).T
            bc = np.asarray(batch[n0 : n0 + n_real], np.float32).reshape(-1, 1)
            batchc.reshape(NSHARD, 1)[:n_real] = bc
        per_core.append(
            dict(
                xT=xT.astype(nbf),
                idxL=np.ascontiguousarray(_wrap16(Lslots.astype(np.int16))),
                idxH=np.ascontiguousarray(_wrap16(Hslots.astype(np.int16))),
                eaT=eaT.astype(nbf),
                dstrel=dstrel,
                batchc=batchc,
            )
        )
    return per_core, C_L, C_H


def _weights(inp):
    """Host-side weight packing (bias folding, concat layouts)."""
    w = {}
    b_in = inp["b_in"].astype(np.float64)
    w["iota128"] = np.broadcast_to(np.arange(128, dtype=np.float32), (128, 128)).copy()
    w["iota32"] = np.broadcast_to(np.arange(32, dtype=np.float32), (128, 32)).copy()
    w["ident"] = np.eye(128, dtype=np.float32).astype(nbf)
    Wg1 = inp["Wg1"].astype(np.float32)
    w["wg1_h1"] = np.concatenate([Wg1[:HID], inp["bg1"].astype(np.float32)[None, :]], 0).astype(nbf)   # [161,160]
    w["wg1_h2"] = np.concatenate([Wg1[HID:], np.zeros((1, HID), np.float32)], 0).astype(nbf)           # [161,160]
    w["wg2rep"] = np.broadcast_to(inp["Wg2"].astype(np.float32)[:, 0], (128, HID)).copy()
    w["wh1"] = np.concatenate([inp["Wh1"].astype(np.float32), inp["bh1"].astype(np.float32)[None, :]], 0).astype(nbf)  # [321,320]
    w["wh2"] = np.concatenate([inp["Wh2"].astype(np.float32), inp["bh2"].astype(np.float32)[None, :]], 0).astype(nbf)  # [321,6]
    w["win"] = inp["W_in"].astype(np.float32).astype(nbf)  # [5,160]
    w["bg2rep"] = np.full((128, 1), float(np.asarray(inp["bg2"]).reshape(-1)[0]), np.float32)
    for layer in range(2):
        Wq, Wk, Wv = (inp[k][layer].astype(np.float64) for k in ("Wq", "Wk", "Wv"))
        bq, bk, bv = (inp[k][layer].astype(np.float64) for k in ("bq", "bk", "bv"))
        Wskip, bskip = inp["Wskip"][layer].astype(np.float64), inp["bskip"][layer].astype(np.float64)
        Wbeta = inp["Wbeta"][layer].astype(np.float64)
        We, be = inp["We"][layer].astype(np.float64), inp["be"][layer].astype(np.float64)
        if layer == 0:
            bq, bk, bv, bskip = bq + b_in @ Wq, bk + b_in @ Wk, bv + b_in @ Wv, bskip + b_in @ Wskip
        P = (Wbeta[:HID, 0] + Wbeta[2 * HID :, 0])
        Q = (Wbeta[HID : 2 * HID, 0] - Wbeta[2 * HID :, 0])
        w[f"wkv{layer}"] = np.concatenate(
            [np.concatenate([Wk, Wv], 1), np.concatenate([bk, bv])[None, :]], 0
        ).astype(np.float32).astype(nbf)                               # [161,320]
        w[f"wq{layer}"] = np.concatenate([Wq, bq[None, :]], 0).astype(np.float32).astype(nbf)  # [161,160]
        rb = np.concatenate([Wskip, (Wskip @ Q)[:, None]], 1)          # [160,161]
        rb_b = np.concatenate([bskip, [bskip @ Q]])[None, :]           # [1,161]
        w[f"wrb{layer}"] = np.concatenate([rb, rb_b], 0).astype(np.float32).astype(nbf)        # [161,161]
        w[f"prep{layer}"] = np.broadcast_to(P.astype(np.float32), (128, HID)).copy()
        w[f"wekv{layer}"] = np.concatenate(
            [np.concatenate([We, We], 1), np.concatenate([be, be])[None, :]], 0
        ).astype(np.float32).astype(nbf)                               # [5,320]
    return w


def _build(C_L, C_H):
    NCH = C_L + C_H
    NGL = (WIN * C_L * 128 + GS - 1) // GS
    NGH = (WIN * C_H * 128 + GS - 1) // GS

    nc = bacc.Bacc(get_trn_type() or "TRN2", target_bir_lowering=False)

    # ---- dram I/O ----
    d = {}
    d["xT"] = nc.dram_tensor("xT", [5, NSHARD], bf16, kind="ExternalInput")
    d["idxL"] = nc.dram_tensor("idxL", [128, NGL * GS // 16], i16, kind="ExternalInput")
    d["idxH"] = nc.dram_tensor("idxH", [128, NGH * GS // 16], i16, kind="ExternalInput")
    d["eaT"] = nc.dram_tensor("eaT", [WIN, 5, NCH * 128], bf16, kind="ExternalInput")
    d["dstrel"] = nc.dram_tensor("dstrel", [WIN, 128, NCH], f32, kind="ExternalInput")
    d["batchc"] = nc.dram_tensor("batchc", [WIN, 128, 1], f32, kind="ExternalInput")
    wshapes = dict(
        iota128=([128, 128], f32), iota32=([128, 32], f32), ident=([128, 128], bf16),
        wg1_h1=([161, HID], bf16), wg1_h2=([161, HID], bf16), wg2rep=([128, HID], f32),
        wh1=([321, JK], bf16), wh2=([321, 6], bf16), win=([5, HID], bf16),
        bg2rep=([128, 1], f32),
    )
    for layer in range(2):
        wshapes[f"wkv{layer}"] = ([161, JK], bf16)
        wshapes[f"wq{layer}"] = ([161, HID], bf16)
        wshapes[f"wrb{layer}"] = ([161, 161], bf16)
        wshapes[f"prep{layer}"] = ([128, HID], f32)
        wshapes[f"wekv{layer}"] = ([5, JK], bf16)
    for k, (shp, dt_) in wshapes.items():
        d[k] = nc.dram_tensor(k, shp, dt_, kind="ExternalInput")
    out_d = nc.dram_tensor("out", [32, 6], f32, kind="ExternalOutput")

    hT = [nc.dram_tensor(f"hT{i}", [HID, NSHARD], bf16) for i in range(3)]
    h_nm = [None, nc.dram_tensor("h_nm1", [NSHARD, HID], bf16),
            nc.dram_tensor("h_nm2", [NSHARD, HID], bf16)]
    kv_own = [nc.dram_tensor(f"kv_own{l}", [NSHARD, KVP], bf16) for l in range(2)]
    kv_full = [nc.dram_tensor(f"kv_full{l}", [NPAD, KVP], bf16, addr_space="Shared")
               for l in range(2)]
    pool_in = nc.dram_tensor("pool_in", [32, JK + 1], f32)
    pool_out = nc.dram_tensor("pool_out", [32, JK + 1], f32, addr_space="Shared")
    rg = [list(range(NCORES))]

    with tile.TileContext(nc) as tc:
        with (
            tc.tile_pool(name="const", bufs=1) as cst,
            tc.tile_pool(name="sb", bufs=2) as sb,
            tc.tile_pool(name="gath", bufs=3) as gath,
            tc.tile_pool(name="ps", bufs=2, space="PSUM") as ps,
        ):
            nc.gpsimd.load_library(mlp)
            regGS = nc.gpsimd.to_reg(GS)

            # ---- persistent constants ----
            C = {}
            def _load_const(key, part, cols, row0=0):
                t = cst.tile([part, cols], wshapes[key][1], name=f"c_{key}_{row0}")
                nc.sync.dma_start(out=t[:], in_=d[key][row0 : row0 + part, :])
                return t
            for layer in range(2):
                C[f"wkv{layer}a"] = _load_const(f"wkv{layer}", 128, JK)
                C[f"wkv{layer}b"] = _load_const(f"wkv{layer}", 32, JK, 128)
                C[f"wkv{layer}c"] = _load_const(f"wkv{layer}", 1, JK, 160)
                C[f"wq{layer}a"] = _load_const(f"wq{layer}", 128, HID)
                C[f"wq{layer}b"] = _load_const(f"wq{layer}", 32, HID, 128)
                C[f"wq{layer}c"] = _load_const(f"wq{layer}", 1, HID, 160)
                C[f"wrb{layer}a"] = _load_const(f"wrb{layer}", 128, 161)
                C[f"wrb{layer}b"] = _load_const(f"wrb{layer}", 32, 161, 128)
                C[f"wrb{layer}c"] = _load_const(f"wrb{layer}", 1, 161, 160)
                C[f"prep{layer}"] = _load_const(f"prep{layer}", 128, HID)
                C[f"wekv{layer}"] = _load_const(f"wekv{layer}", 5, JK)
            C["iota128"] = _load_const("iota128", 128, 128)
            C["iota32"] = _load_const("iota32", 128, 32)
            C["ident"] = _load_const("ident", 128, 128)
            C["wg2rep"] = _load_const("wg2rep", 128, HID)
            for key in ("wg1_h1", "wg1_h2"):
                C[key + "a"] = _load_const(key, 128, HID)
                C[key + "b"] = _load_const(key, 32, HID, 128)
                C[key + "c"] = _load_const(key, 1, HID, 160)
            C["bg2rep"] = _load_const("bg2rep", 128, 1)
            C["wh1a"] = _load_const("wh1", 128, JK)
            C["wh1b"] = _load_const("wh1", 128, JK, 128)
            C["wh1c"] = _load_const("wh1", 64, JK, 256)
            C["wh1d"] = _load_const("wh1", 1, JK, 320)
            C["wh2a"] = _load_const("wh2", 128, 6)
            C["wh2b"] = _load_const("wh2", 128, 6, 128)
            C["wh2c"] = _load_const("wh2", 64, 6, 256)
            C["wh2d"] = _load_const("wh2", 1, 6, 320)
            C["win"] = _load_const("win", 5, HID)

            idxLt = cst.tile([128, NGL * GS // 16], i16, name="idxLt")
            nc.sync.dma_start(out=idxLt[:], in_=d["idxL"][:])
            idxHt = cst.tile([128, NGH * GS // 16], i16, name="idxHt")
            nc.sync.dma_start(out=idxHt[:], in_=d["idxH"][:])

            # ---- phase 0: h0T = (x @ W_in)^T, own nodes ----
            with nc.named_scope("p0"):
                NT0 = (NSHARD + 511) // 512
                for t in range(NT0):
                    c0, cn = t * 512, min(512, NSHARD - t * 512)
                    xts = sb.tile([5, cn], bf16, tag="xts")
                    nc.sync.dma_start(out=xts[:], in_=d["xT"][:, c0 : c0 + cn])
                    for (r0, m) in ((0, 128), (128, 32)):
                        ph = ps.tile([m, cn], f32, tag="kve", bufs=3)
                        nc.tensor.matmul(ph[:], C["win"][:, r0 : r0 + m], xts[:],
                                         start=True, stop=True)
                        hsb = sb.tile([m, cn], bf16, tag="hsb")
                        nc.vector.tensor_copy(out=hsb[:], in_=ph[:])
                        nc.sync.dma_start(out=hT[0][r0 : r0 + m, c0 : c0 + cn], in_=hsb[:])

            ones1 = cst.tile([1, 128], bf16, name="ones1")
            nc.gpsimd.memset(ones1[:], 1.0)

            # ==== two layers ====
            for layer in range(2):
                hsrc = hT[layer]
                # ---- kv GEMM own nodes -> kv_own ----
                with nc.named_scope(f"kv{layer}"):
                    for t in range(WIN):
                        csl = slice(t * 128, (t + 1) * 128)
                        hta = sb.tile([128, 128], bf16, tag="hta", bufs=3)
                        nc.sync.dma_start(out=hta[:], in_=hsrc[0:128, csl])
                        htb = sb.tile([32, 128], bf16, tag="htb", bufs=3)
                        nc.sync.dma_start(out=htb[:], in_=hsrc[128:160, csl])
                        pkv = ps.tile([128, JK], f32, tag="kve", bufs=3)
                        nc.tensor.matmul(pkv[:], hta[:], C[f"wkv{layer}a"][:], start=True, stop=False)
                        nc.tensor.matmul(pkv[:], htb[:], C[f"wkv{layer}b"][:], start=False, stop=False)
                        nc.tensor.matmul(pkv[:], ones1[:, :128], C[f"wkv{layer}c"][:], start=False, stop=True)
                        kvsb = sb.tile([128, JK], bf16, tag="kvsb")
                        nc.vector.tensor_copy(out=kvsb[:], in_=pkv[:])
                        nc.sync.dma_start(out=kv_own[layer][csl, 0:JK], in_=kvsb[:])
                with nc.named_scope(f"ag{layer}"):
                    nc.gpsimd.collective_compute(
                        "AllGather", mybir.AluOpType.bypass, replica_groups=rg,
                        ins=[kv_own[layer][:]], outs=[kv_full[layer][:]])

                # ---- edge phase ----
                with nc.named_scope(f"edge{layer}"):
                    cur = {"L": -1, "H": -1}
                    cur_tile = {"L": None, "H": None}

                    def _gather(region, gt):
                        if cur[region] == gt:
                            return cur_tile[region]
                        idxt, base, ng = (
                            (idxLt, 0, NGL) if region == "L" else (idxHt, SPLIT, NGH)
                        )
                        gtile = gath.tile([128, GS // 128, KVP], bf16, tag="g" + region)
                        nc.gpsimd.dma_gather(
                            gtile[:],
                            kv_full[layer][base : base + SPLIT, :],
                            idxt[:, gt * (GS // 16) : (gt + 1) * (GS // 16)],
                            num_idxs=GS, num_idxs_reg=regGS, elem_size=KVP)
                        cur[region] = gt
                        cur_tile[region] = gtile
                        return gtile

                    for w in range(WIN):
                        wsl = slice(w * 128, (w + 1) * 128)
                        eaw = sb.tile([5, NCH * 128], bf16, tag="eaw", bufs=3)
                        nc.sync.dma_start(out=eaw[:], in_=d["eaT"][w])
                        drw = sb.tile([128, NCH], f32, tag="drw", bufs=3)
                        nc.sync.dma_start(out=drw[:], in_=d["dstrel"][w])
                        hta = sb.tile([128, 128], bf16, tag="hta", bufs=3)
                        nc.sync.dma_start(out=hta[:], in_=hsrc[0:128, wsl])
                        htb = sb.tile([32, 128], bf16, tag="htb", bufs=3)
                        nc.sync.dma_start(out=htb[:], in_=hsrc[128:160, wsl])
                        # q for this window
                        pq = ps.tile([128, HID], f32, tag="qrb", bufs=1)
                        nc.tensor.matmul(pq[:], hta[:], C[f"wq{layer}a"][:], start=True, stop=False)
                        nc.tensor.matmul(pq[:], htb[:], C[f"wq{layer}b"][:], start=False, stop=False)
                        nc.tensor.matmul(pq[:], ones1[:, :128], C[f"wq{layer}c"][:], start=False, stop=True)
                        qw = sb.tile([128, HID], bf16, tag="qw", bufs=3)
                        nc.vector.tensor_copy(out=qw[:], in_=pq[:])
                        # r / beta-partial for this window
                        prb = ps.tile([128, 161], f32, tag="qrb", bufs=1)
                        nc.tensor.matmul(prb[:], hta[:], C[f"wrb{layer}a"][:], start=True, stop=False)
                        nc.tensor.matmul(prb[:], htb[:], C[f"wrb{layer}b"][:], start=False, stop=False)
                        nc.tensor.matmul(prb[:], ones1[:, :128], C[f"wrb{layer}c"][:], start=False, stop=True)
                        rsb = sb.tile([128, 161], f32, tag="rsb", bufs=3)
                        nc.vector.tensor_copy(out=rsb[:], in_=prb[:])

                        pacc = ps.tile([128, 164], f32, tag="acc", bufs=1)
                        for j in range(NCH):
                            if j < C_L:
                                cidx = w * C_L + j
                                gtile = _gather("L", cidx // (GS // 128))
                            else:
                                cidx = w * C_H + (j - C_L)
                                gtile = _gather("H", cidx // (GS // 128))
                            sub = cidx % (GS // 128)
                            kv_g = gtile[:, sub, 0:JK]
                            # e = ea @ We (+bias) in PSUM
                            pe = ps.tile([128, JK], f32, tag="kve", bufs=3)
                            nc.tensor.matmul(pe[:], eaw[:, j * 128 : (j + 1) * 128],
                                             C[f"wekv{layer}"][:], start=True, stop=True)
                            # kv_e = kv_g + e
                            kve = sb.tile([128, JK], f32, tag="kvesb", bufs=4)
                            nc.vector.tensor_tensor(out=kve[:], in0=pe[:], in1=kv_g,
                                                    op=mybir.AluOpType.add)
                            # S^T one-hot [edges, nodes]
                            st = sb.tile([128, 128], bf16, tag="st", bufs=4)
                            nc.vector.tensor_tensor(
                                out=st[:], in0=drw[:, j : j + 1].to_broadcast([128, 128]),
                                in1=C["iota128"][:], op=mybir.AluOpType.is_equal)
                            # S = (S^T)^T via PE transpose
                            pst = ps.tile([128, 128], bf16, tag="tp")
                            nc.tensor.transpose(pst[:], st[:], C["ident"][:])
                            ssb = sb.tile([128, 128], bf16, tag="ssb", bufs=4)
                            nc.vector.tensor_copy(out=ssb[:], in_=pst[:])
                            # q gathered to edges
                            pqg = ps.tile([128, HID], f32, tag="tp2", bufs=1)
                            nc.tensor.matmul(pqg[:], ssb[:], qw[:], start=True, stop=True)
                            # alpha = sum_d q_g * k_e per head
                            tq = sb.tile([128, HID], f32, tag="tq", bufs=4)
                            nc.vector.tensor_tensor(out=tq[:], in0=pqg[:], in1=kve[:, :HID],
                                                    op=mybir.AluOpType.mult)
                            al = sb.tile([128, H], f32, tag="al", bufs=4)
                            nc.vector.tensor_reduce(
                                out=al[:], in_=tq[:].rearrange("p (h dd) -> p h dd", h=H),
                                axis=mybir.AxisListType.X, op=mybir.AluOpType.add)
                            # w tile: [v_e * ex | ex]
                            wt = sb.tile([128, 164], bf16, tag="wt", bufs=4)
                            nc.scalar.activation(out=wt[:, 160:164], in_=al[:],
                                                 func=mybir.ActivationFunctionType.Exp,
                                                 scale=INVSQD)
                            nc.vector.tensor_tensor(
                                out=wt[:, :HID].rearrange("p (h dd) -> p h dd", h=H),
                                in0=kve[:, HID:].rearrange("p (h dd) -> p h dd", h=H),
                                in1=wt[:, 160:164].rearrange("p (h o) -> p h o", h=H).to_broadcast([128, H, D]),
                                op=mybir.AluOpType.mult)
                            # scatter: acc[nodes] += S^T.T @ [w | ex]
                            nc.tensor.matmul(pacc[:], st[:], wt[:],
                                             start=(j == 0), stop=(j == NCH - 1),
                                             skip_group_check=True)

                        # ---- window post: out = num/den, beta gate, h' ----
                        accsb = sb.tile([128, 164], f32, tag="accsb")
                        nc.vector.tensor_copy(out=accsb[:], in_=pacc[:])
                        dmax = sb.tile([128, H], f32, tag="dmax")
                        nc.vector.tensor_scalar_max(dmax[:], accsb[:, 160:164], 1e-30)
                        denr = sb.tile([128, H], f32, tag="denr")
                        nc.vector.reciprocal(out=denr[:], in_=dmax[:])
                        outn = sb.tile([128, HID], f32, tag="outn")
                        nc.vector.tensor_tensor(
                            out=outn[:].rearrange("p (h dd) -> p h dd", h=H),
                            in0=accsb[:, :HID].rearrange("p (h dd) -> p h dd", h=H),
                            in1=denr[:].rearrange("p (h o) -> p h o", h=H).to_broadcast([128, H, D]),
                            op=mybir.AluOpType.mult)
                        scr = sb.tile([128, HID], f32, tag="scr")
                        outP = sb.tile([128, 1], f32, tag="outP")
                        nc.vector.tensor_tensor(out=scr[:], in0=outn[:],
                            in1=C[f"prep{layer}"][:], op=mybir.AluOpType.mult)
                        nc.vector.tensor_reduce(out=outP[:],
                            in_=scr[:].rearrange("p (a b) -> p a b", a=1),
                            axis=mybir.AxisListType.XY, op=mybir.AluOpType.add)
                        beta = sb.tile([128, 1], f32, tag="beta")
                        nc.scalar.activation(out=beta[:], in_=outP[:],
                                             func=mybir.ActivationFunctionType.Sigmoid,
                                             bias=rsb[:, 160:161], scale=1.0)
                        dvec = sb.tile([128, HID], f32, tag="dvec")
                        nc.vector.tensor_sub(dvec[:], rsb[:, :HID], outn[:])
                        hp = sb.tile([128, HID], bf16, tag="hp")
                        nc.vector.scalar_tensor_tensor(
                            out=hp[:], in0=dvec[:], scalar=beta[:, 0:1], in1=outn[:],
                            op0=mybir.AluOpType.mult, op1=mybir.AluOpType.add)
                        nc.sync.dma_start(out=h_nm[layer + 1][wsl, :], in_=hp[:])
                        # transpose h' into hT[layer+1]
                        ptr1 = ps.tile([128, 128], bf16, tag="tp")
                        nc.tensor.transpose(ptr1[:], hp[:, 0:128], C["ident"][:])
                        t1 = sb.tile([128, 128], bf16, tag="t1")
                        nc.vector.tensor_copy(out=t1[:], in_=ptr1[:])
                        nc.sync.dma_start(out=hT[layer + 1][0:128, wsl], in_=t1[:])
                        ptr2 = ps.tile([32, 128], bf16, tag="tp")
                        nc.tensor.transpose(ptr2[:], hp[:, 128:160], C["ident"][:])
                        t2 = sb.tile([32, 128], bf16, tag="t2")
                        nc.vector.tensor_copy(out=t2[:], in_=ptr2[:])
                        nc.sync.dma_start(out=hT[layer + 1][128:160, wsl], in_=t2[:])

            # ==== final phase: gate + graph pooling + head MLP ====
            with nc.named_scope("final"):
                pgr = ps.tile([32, JK + 1], f32, tag="acc", bufs=1)
                for w in range(WIN):
                    wsl = slice(w * 128, (w + 1) * 128)
                    h1w = sb.tile([128, HID], bf16, tag="h1w")
                    nc.sync.dma_start(out=h1w[:], in_=h_nm[1][wsl, :])
                    h2w = sb.tile([128, HID], bf16, tag="h2w")
                    nc.sync.dma_start(out=h2w[:], in_=h_nm[2][wsl, :])
                    bcw = sb.tile([128, 1], f32, tag="bcw")
                    nc.sync.dma_start(out=bcw[:], in_=d["batchc"][w])
                    pg = ps.tile([128, HID], f32, tag="kve", bufs=3)
                    first = True
                    for (src_hT, wkey) in ((hT[1], "wg1_h1"), (hT[2], "wg1_h2")):
                        g_a = sb.tile([128, 128], bf16, tag="hta", bufs=3)
                        nc.sync.dma_start(out=g_a[:], in_=src_hT[0:128, wsl])
                        g_b = sb.tile([32, 128], bf16, tag="htb", bufs=3)
                        nc.sync.dma_start(out=g_b[:], in_=src_hT[128:160, wsl])
                        nc.tensor.matmul(pg[:], g_a[:], C[wkey + "a"][:], start=first, stop=False)
                        first = False
                        nc.tensor.matmul(pg[:], g_b[:], C[wkey + "b"][:], start=False, stop=False)
                    nc.tensor.matmul(pg[:], ones1[:, :128], C["wg1_h1c"][:], start=False, stop=True)
                    grelu = sb.tile([128, HID], f32, tag="grelu")
                    nc.scalar.activation(out=grelu[:], in_=pg[:],
                                         func=mybir.ActivationFunctionType.Relu)
                    scr2 = sb.tile([128, HID], f32, tag="scr")
                    gatec = sb.tile([128, 1], f32, tag="gatec")
                    nc.vector.tensor_tensor(out=scr2[:], in0=grelu[:],
                        in1=C["wg2rep"][:], op=mybir.AluOpType.mult)
                    nc.vector.tensor_reduce(out=gatec[:],
                        in_=scr2[:].rearrange("p (a b) -> p a b", a=1),
                        axis=mybir.AxisListType.XY, op=mybir.AluOpType.add)
                    ge = sb.tile([128, 1], f32, tag="ge")
                    nc.scalar.activation(out=ge[:], in_=gatec[:],
                                         func=mybir.ActivationFunctionType.Exp,
                                         bias=C["bg2rep"][:, 0:1])
                    sg = sb.tile([128, 32], bf16, tag="sg")
                    nc.vector.tensor_tensor(out=sg[:], in0=bcw[:].to_broadcast([128, 32]),
                                            in1=C["iota32"][:], op=mybir.AluOpType.is_equal)
                    wg = sb.tile([128, JK + 1], bf16, tag="wg")
                    nc.vector.tensor_scalar_mul(wg[:, 0:HID], h1w[:], ge[:, 0:1])
                    nc.vector.tensor_scalar_mul(wg[:, HID:JK], h2w[:], ge[:, 0:1])
                    nc.vector.tensor_copy(out=wg[:, JK : JK + 1], in_=ge[:])
                    nc.tensor.matmul(pgr[:], sg[:], wg[:], start=(w == 0),
                                     stop=(w == WIN - 1), skip_group_check=True)
                pg_sb = sb.tile([32, JK + 1], f32, tag="pg_sb")
                nc.vector.tensor_copy(out=pg_sb[:], in_=pgr[:])
                nc.sync.dma_start(out=pool_in[:], in_=pg_sb[:])
                nc.gpsimd.collective_compute(
                    "AllReduce", mybir.AluOpType.add, replica_groups=rg,
                    ins=[pool_in[:]], outs=[pool_out[:]])
                psb = sb.tile([32, JK + 1], f32, tag="psb")
                nc.sync.dma_start(out=psb[:], in_=pool_out[:])
                gden = sb.tile([32, 1], f32, tag="gden")
                nc.vector.tensor_scalar_max(gden[:], psb[:, JK : JK + 1], 1e-30)
                gdr = sb.tile([32, 1], f32, tag="gdr")
                nc.vector.reciprocal(out=gdr[:], in_=gden[:])
                pl = sb.tile([32, JK], bf16, tag="pl")
                nc.vector.tensor_scalar_mul(pl[:], psb[:, 0:JK], gdr[:, 0:1])

                def _headmm(vin, wa, wb, wc, wd, nout, tagp):
                    """vin [32, 320] @ W[320, nout] + bias via PE transposes."""
                    pouts = ps.tile([32, nout], f32, tag=tagp, bufs=(3 if tagp == "kve" else 1))
                    for si, (c0, m) in enumerate(((0, 128), (128, 128), (256, 64))):
                        ptt = ps.tile([m, 32], bf16, tag="tp")
                        nc.tensor.transpose(ptt[:], vin[:, c0 : c0 + m], C["ident"][0:32, 0:32])
                        tsb = sb.tile([m, 32], bf16, tag="tsb")
                        nc.vector.tensor_copy(out=tsb[:], in_=ptt[:])
                        nc.tensor.matmul(pouts[:], tsb[:], (wa, wb, wc)[si][:m, :],
                                         start=(si == 0), stop=False, skip_group_check=True)
                    nc.tensor.matmul(pouts[:], ones1[:, :32], wd[:],
                                     start=False, stop=True, skip_group_check=True)
                    return pouts

                ph1 = _headmm(pl, C["wh1a"], C["wh1b"], C["wh1c"], C["wh1d"], JK, "qrb")
                vrel = sb.tile([32, JK], bf16, tag="vrel")
                nc.scalar.activation(out=vrel[:], in_=ph1[:],
                                     func=mybir.ActivationFunctionType.Relu)
                ph2 = _headmm(vrel, C["wh2a"], C["wh2b"], C["wh2c"], C["wh2d"], 6, "kve")
                osb = sb.tile([32, 6], f32, tag="osb")
                nc.vector.tensor_copy(out=osb[:], in_=ph2[:])
                nc.sync.dma_start(out=out_d[:], in_=osb[:])

    nc.compile()
    return nc


_CACHE = {}
_LAST_RES = None


def kernel(**inputs):
    inputs = {k: np.asarray(v) for k, v in inputs.items()}
    per_core, C_L, C_H = _preprocess(
        inputs["x"], inputs["edge_index"], inputs["edge_attr"], inputs["batch"])
    w = _weights(inputs)
    key = (C_L, C_H)
    if key not in _CACHE:
        _CACHE[key] = _build(C_L, C_H)
    nc = _CACHE[key]
    in_maps = []
    for r in range(NCORES):
        m = dict(w)
        m.update(per_core[r])
        in_maps.append(m)
    import os
    trace = bool(os.environ.get("KERNEL_TRACE"))
    if trace:
        try:
            import axon_prof
            axon_prof.install()
        except Exception:
            trace = False
    res = run_bass_kernel_spmd(nc, in_maps, core_ids=list(range(NCORES)), trace=trace)
    if trace and res.exec_time_ns is not None:
        print(f"HW exec time: {res.exec_time_ns} ns")
        if res.per_core_scope_times:
            for scope, cores in sorted(res.per_core_scope_times.items()):
                print(f"  scope {scope}: {cores}")
    global _LAST_RES
    _LAST_RES = res
    out = res.results[0]["out"]
    return out.reshape(G, 2, 3).astype(np.float32)


# revision 11
# speedup vs baseline: 2.7577x; 1.5345x over previous
"""Trainium2 Bass kernel for nn_EndpointRegressor (2x TransformerConv GNN +
AttentionalAggregation) distributed over 8 NeuronCores.

Sharding: edges partitioned by destination node range (6272 nodes/core);
each core owns its dst nodes exclusively, so segment softmax/scatter stats
need no cross-core reduction.

Layer 0 is gather-free: k0/v0 are low-rank in host-known inputs
(k0[src] = x[src]@(W_in Wk) + ea@We + bias), so alpha0 = sum_c attr10[e,c] *
qW0[dst,h,c] with attr10 = [x[src](5), ea(4), 1] riding the edge stream, and
the value scatter accumulates T[dst,h,c] = sum_e ex*attr10 which is expanded
to 160 dims by one small matmul per 128-node window.  No kv GEMM, no
AllGather, no gather for layer 0.

Layer 1 computes kv for own nodes, AllGathers the [50176,320] bf16 table,
and gathers per-edge rows with indirect_dma_start (int32 indices, HW DGE).
The edge-feature term is folded the same way (qWe trick) so no per-chunk
e-matmul.  Per 128-edge chunk only two PE matmuls remain (q-gather via
one-hot, scatter via one-hot); all per-edge vector math is batched per
128-dst-node window.  Segment softmax uses exp without max subtraction
(alpha ~ +-0.1 for this model family); division by the denominator happens
on the node side.  All matmul operands bf16, PSUM accumulation fp32.
"""
import math
import numpy as np
import ml_dtypes

import concourse.bass as bass
import concourse.bacc as bacc
import concourse.mybir as mybir
import concourse.tile as tile
from concourse._compat import get_trn_type
from concourse.bass_utils import run_bass_kernel_spmd
from concourse.library_config import mlp

# ---- problem constants (fixed by the problem spec) ----
N, E, G = 50000, 500000, 32
H, D = 4, 40
HID = H * D            # 160
JK = 2 * HID           # 320
NCORES = 8
NSHARD = 6272          # 49*128 nodes per core
NPAD = NCORES * NSHARD # 50176
WIN = NSHARD // 128    # 49
INVSQD = 1.0 / math.sqrt(float(D))

f32 = mybir.dt.float32
bf16 = mybir.dt.bfloat16
i32 = mybir.dt.int32
nbf = ml_dtypes.bfloat16


def _preprocess(x, edge_index, edge_attr, batch):
    """Sort edges by dst, shard by dst range, pad each (core,window) bucket
    to NCH chunks of 128 edge slots. Build the per-edge attribute streams."""
    src = np.asarray(edge_index[0], dtype=np.int64)
    dst = np.asarray(edge_index[1], dtype=np.int64)
    ea = np.asarray(edge_attr, dtype=np.float32)
    x = np.asarray(x, np.float32)
    order = np.argsort(dst, kind="stable")
    src, dst, ea = src[order], dst[order], ea[order]

    core = dst // NSHARD
    win = (dst % NSHARD) // 128

    buckets = {}
    for r in range(NCORES):
        m_r = core == r
        for w in range(WIN):
            buckets[(r, w)] = np.nonzero(m_r & (win == w))[0]

    NCH = max(1, max((len(b) + 127) // 128 for b in buckets.values()))

    per_core = []
    for r in range(NCORES):
        idxE = np.zeros((128, WIN * NCH), np.int32)
        attr0 = np.zeros((128, WIN * NCH * 40), np.float32)
        eaQ1 = np.zeros((128, WIN * NCH * 20), np.float32)
        drwC = np.full((128, WIN * NCH), -1.0, np.float32)
        drwR = np.full((WIN, 1, NCH * 128), -1.0, np.float32)
        for w in range(WIN):
            eidx = buckets[(r, w)]
            n = len(eidx)
            kk = np.arange(n)
            jj = kk // 128
            pp = kk % 128
            col = w * NCH + jj
            idxE[pp, col] = src[eidx]
            dd = (dst[eidx] % 128).astype(np.float32)
            drwC[pp, col] = dd
            drwR[w, 0, jj * 128 + pp] = dd
            # attr10 = [x[src](5), ea(4), 1] replicated per head
            a10 = np.concatenate(
                [x[src[eidx]], ea[eidx], np.ones((n, 1), np.float32)], 1)  # [n,10]
            ea5 = np.concatenate([ea[eidx], np.ones((n, 1), np.float32)], 1)  # [n,5]
            for h in range(H):
                c0 = col * 40 + h * 10
                for c in range(10):
                    attr0[pp, c0 + c] = a10[:, c]
                c1 = col * 20 + h * 5
                for c in range(5):
                    eaQ1[pp, c1 + c] = ea5[:, c]
        n0 = r * NSHARD
        xT6 = np.zeros((6, NSHARD), np.float32)
        xT6[5, :] = 1.0
        batchc = np.full((128, WIN), -1.0, np.float32)
        n_real = max(0, min(NSHARD, N - n0))
        if n_real > 0:
            xT6[:5, :n_real] = x[n0 : n0 + n_real].T
            bfull = np.full(NSHARD, -1.0, np.float32)
            bfull[:n_real] = np.asarray(batch[n0 : n0 + n_real], np.float32)
            batchc[:, :] = bfull.reshape(WIN, 128).T
        per_core.append(
            dict(
                xT6=xT6.astype(nbf),
                idxE=idxE,
                attr0=attr0.astype(nbf),
                eaQ1=eaQ1.astype(nbf),
                drwC=drwC,
                drwR=drwR,
                batchc=batchc,
            )
        )
    return per_core, NCH


def _weights(inp):
    """Host-side weight packing/folding (fp64 math, bf16 output)."""
    w = {}
    f8 = np.float64
    W_in = inp["W_in"].astype(f8)
    b_in = inp["b_in"].astype(f8)

    w["iota128"] = np.broadcast_to(np.arange(128, dtype=np.float32), (128, 128)).copy()
    w["iota32"] = np.broadcast_to(np.arange(32, dtype=np.float32), (128, 32)).copy()
    w["iotaP"] = np.arange(128, dtype=np.float32).reshape(128, 1).copy()
    w["ident"] = np.eye(128, dtype=np.float32).astype(nbf)
    w["identf"] = np.eye(128, dtype=np.float32)
    Wg1 = inp["Wg1"].astype(np.float32)
    w["wg1_h1"] = np.concatenate([Wg1[:HID], inp["bg1"].astype(np.float32)[None, :]], 0).astype(nbf)
    w["wg1_h2"] = np.concatenate([Wg1[HID:], np.zeros((1, HID), np.float32)], 0).astype(nbf)
    w["wg2rep"] = np.broadcast_to(inp["Wg2"].astype(np.float32)[:, 0], (128, HID)).copy()
    w["wh1"] = np.concatenate([inp["Wh1"].astype(np.float32), inp["bh1"].astype(np.float32)[None, :]], 0).astype(nbf)
    w["wh2"] = np.concatenate([inp["Wh2"].astype(np.float32), inp["bh2"].astype(np.float32)[None, :]], 0).astype(nbf)
    w["bg2rep"] = np.full((128, 1), float(np.asarray(inp["bg2"]).reshape(-1)[0]), np.float32)

    for layer in range(2):
        Wq, Wk, Wv = (inp[k][layer].astype(f8) for k in ("Wq", "Wk", "Wv"))
        bq, bk, bv = (inp[k][layer].astype(f8) for k in ("bq", "bk", "bv"))
        Wskip, bskip = inp["Wskip"][layer].astype(f8), inp["bskip"][layer].astype(f8)
        Wbeta = inp["Wbeta"][layer].astype(f8)
        We, be = inp["We"][layer].astype(f8), inp["be"][layer].astype(f8)
        P = (Wbeta[:HID, 0] + Wbeta[2 * HID :, 0])
        Q = (Wbeta[HID : 2 * HID, 0] - Wbeta[2 * HID :, 0])
        w[f"prep{layer}"] = np.broadcast_to(P.astype(np.float32), (128, HID)).copy()
        if layer == 0:
            Q6 = np.concatenate([W_in @ Wq, (b_in @ Wq + bq)[None, :]], 0)     # [6,160]
            K10 = np.concatenate([W_in @ Wk, We, (b_in @ Wk + bk + be)[None, :]], 0)  # [10,160]
            V10 = np.concatenate([W_in @ Wv, We, (b_in @ Wv + bv + be)[None, :]], 0)  # [10,160]
            wq0x = np.zeros((6, 40), f8)
            wvblk0 = np.zeros((40, HID), f8)
            for h in range(H):
                ds = slice(h * D, (h + 1) * D)
                wq0x[:, h * 10 : (h + 1) * 10] = np.einsum(
                    "fd,cd->fc", Q6[:, ds], K10[:, ds])
                wvblk0[h * 10 : (h + 1) * 10, ds] = V10[:, ds]
            S6 = np.concatenate([W_in @ Wskip, (b_in @ Wskip + bskip)[None, :]], 0)  # [6,160]
            wrb0 = np.concatenate([S6, -(S6 @ Q)[:, None]], 1)            # [6,161]
            w["wq0x"] = wq0x.astype(np.float32).astype(nbf)
            w["wvblk0"] = wvblk0.astype(np.float32).astype(nbf)
            w["wrb0"] = wrb0.astype(np.float32).astype(nbf)
        else:
            We5 = np.concatenate([We, be[None, :]], 0)                    # [5,160]
            Q161 = np.concatenate([Wq, bq[None, :]], 0)                   # [161,160]
            wq1x = np.zeros((161, 180), f8)
            weblk1 = np.zeros((20, HID), f8)
            for h in range(H):
                ds = slice(h * D, (h + 1) * D)
                wq1x[:, h * 45 : h * 45 + 40] = Q161[:, ds]
                wq1x[:, h * 45 + 40 : h * 45 + 45] = np.einsum(
                    "fd,cd->fc", Q161[:, ds], We5[:, ds])
                weblk1[h * 5 : (h + 1) * 5, ds] = We5[:, ds]
            S161 = np.concatenate([Wskip, bskip[None, :]], 0)             # [161,160]
            wrb1 = np.concatenate([S161, -(S161 @ Q)[:, None]], 1)        # [161,161]
            wkv1 = np.concatenate(
                [np.concatenate([Wk, Wv], 1), np.concatenate([bk, bv])[None, :]], 0)  # [161,320]
            w["wq1x"] = wq1x.astype(np.float32).astype(nbf)
            w["weblk1"] = weblk1.astype(np.float32).astype(nbf)
            w["wrb1"] = wrb1.astype(np.float32).astype(nbf)
            w["wkv1"] = wkv1.astype(np.float32).astype(nbf)
    return w


def _build(NCH, phases="full"):
    nc = bacc.Bacc(get_trn_type() or "TRN2", target_bir_lowering=False)

    # ---- dram I/O ----
    d = {}
    d["xT6"] = nc.dram_tensor("xT6", [6, NSHARD], bf16, kind="ExternalInput")
    d["idxE"] = nc.dram_tensor("idxE", [128, WIN * NCH], i32, kind="ExternalInput")
    d["attr0"] = nc.dram_tensor("attr0", [128, WIN * NCH * 40], bf16, kind="ExternalInput")
    d["eaQ1"] = nc.dram_tensor("eaQ1", [128, WIN * NCH * 20], bf16, kind="ExternalInput")
    d["drwC"] = nc.dram_tensor("drwC", [128, WIN * NCH], f32, kind="ExternalInput")
    d["drwR"] = nc.dram_tensor("drwR", [WIN, 1, NCH * 128], f32, kind="ExternalInput")
    d["batchc"] = nc.dram_tensor("batchc", [128, WIN], f32, kind="ExternalInput")
    wshapes = dict(
        iota128=([128, 128], f32), iota32=([128, 32], f32), iotaP=([128, 1], f32),
        ident=([128, 128], bf16), identf=([128, 128], f32),
        wg1_h1=([161, HID], bf16), wg1_h2=([161, HID], bf16), wg2rep=([128, HID], f32),
        wh1=([321, JK], bf16), wh2=([321, 6], bf16), bg2rep=([128, 1], f32),
        prep0=([128, HID], f32), prep1=([128, HID], f32),
        wq0x=([6, 40], bf16), wvblk0=([40, HID], bf16), wrb0=([6, 161], bf16),
        wq1x=([161, 180], bf16), weblk1=([20, HID], bf16), wrb1=([161, 161], bf16),
        wkv1=([161, JK], bf16),
    )
    for k, (shp, dt_) in wshapes.items():
        d[k] = nc.dram_tensor(k, shp, dt_, kind="ExternalInput")
    out_d = nc.dram_tensor("out", [32, 6], f32, kind="ExternalOutput")

    hT = [nc.dram_tensor(f"hT{i}", [HID, NSHARD], bf16) for i in (1, 2)]  # [0]->h1T, [1]->h2T
    h_nm = [nc.dram_tensor(f"h_nm{i}", [NSHARD, HID], bf16) for i in (1, 2)]
    kv_own = nc.dram_tensor("kv_own1", [NSHARD, JK], bf16)
    kv_full = nc.dram_tensor("kv_full1", [NPAD, JK], bf16, addr_space="Shared")
    pool_in = nc.dram_tensor("pool_in", [32, JK + 1], f32)
    pool_out = nc.dram_tensor("pool_out", [32, JK + 1], f32, addr_space="Shared")
    rg = [list(range(NCORES))]

    with tile.TileContext(nc) as tc:
        with (
            tc.tile_pool(name="cst", bufs=1) as cst,
            tc.tile_pool(name="sb", bufs=2) as sb,
            tc.tile_pool(name="gath", bufs=3) as gath,
            tc.tile_pool(name="ps", bufs=2, space="PSUM") as ps,
        ):
            nc.gpsimd.load_library(mlp)

            # ---- persistent constants + preloads ----
            C = {}
            def _load_const(key, part, cols, row0=0):
                t = cst.tile([part, cols], wshapes[key][1], name=f"c_{key}_{row0}")
                nc.sync.dma_start(out=t[:], in_=d[key][row0 : row0 + part, :])
                return t
            for key in ("iota128", "iota32", "iotaP", "ident", "identf", "wg2rep",
                        "bg2rep", "prep0", "prep1", "wvblk0", "weblk1"):
                C[key] = _load_const(key, wshapes[key][0][0], wshapes[key][0][1])
            C["wq0x"] = _load_const("wq0x", 6, 40)
            C["wrb0"] = _load_const("wrb0", 6, 161)
            for key, cols in (("wq1x", 180), ("wrb1", 161), ("wkv1", JK),
                              ("wg1_h1", HID), ("wg1_h2", HID)):
                C[key + "a"] = _load_const(key, 128, cols)
                C[key + "b"] = _load_const(key, 32, cols, 128)
                C[key + "c"] = _load_const(key, 1, cols, 160)
            C["wh1a"] = _load_const("wh1", 128, JK)
            C["wh1b"] = _load_const("wh1", 128, JK, 128)
            C["wh1c"] = _load_const("wh1", 64, JK, 256)
            C["wh1d"] = _load_const("wh1", 1, JK, 320)
            C["wh2a"] = _load_const("wh2", 128, 6)
            C["wh2b"] = _load_const("wh2", 128, 6, 128)
            C["wh2c"] = _load_const("wh2", 64, 6, 256)
            C["wh2d"] = _load_const("wh2", 1, 6, 320)

            idxt = cst.tile([128, WIN * NCH], i32, name="idxt")
            nc.sync.dma_start(out=idxt[:], in_=d["idxE"][:])
            attr0t = cst.tile([128, WIN * NCH * 40], bf16, name="attr0t")
            nc.sync.dma_start(out=attr0t[:], in_=d["attr0"][:])
            eaQ1t = cst.tile([128, WIN * NCH * 20], bf16, name="eaQ1t")
            nc.sync.dma_start(out=eaQ1t[:], in_=d["eaQ1"][:])
            drwCt = cst.tile([128, WIN * NCH], f32, name="drwCt")
            nc.sync.dma_start(out=drwCt[:], in_=d["drwC"][:])
            batchct = cst.tile([128, WIN], f32, name="batchct")
            nc.sync.dma_start(out=batchct[:], in_=d["batchc"][:])

            ones1 = cst.tile([1, 128], bf16, name="ones1")
            nc.gpsimd.memset(ones1[:], 1.0)

            NE = NCH * 128

            def _onehots(w):
                """stw [128, NCH*128] (edges p -> node cols), sttw (nodes p ->
                edge cols) one-hot masks for window w, both bf16."""
                drw_r = sb.tile([1, NE], f32, tag="drwr", bufs=2)
                nc.sync.dma_start(out=drw_r[:], in_=d["drwR"][w])
                stw = sb.tile([128, NE], bf16, tag="stw")
                nc.vector.tensor_tensor(
                    out=stw[:].rearrange("p (c x) -> p c x", x=128),
                    in0=drwCt[:, w * NCH : (w + 1) * NCH]
                        .rearrange("p (c o) -> p c o", o=1)
                        .to_broadcast([128, NCH, 128]),
                    in1=C["iota128"][:].rearrange("p (o x) -> p o x", o=1)
                        .to_broadcast([128, NCH, 128]),
                    op=mybir.AluOpType.is_equal)
                rep = sb.tile([128, NE], f32, tag="rep")
                nc.gpsimd.partition_broadcast(rep[:], drw_r[:])
                sttw = sb.tile([128, NE], bf16, tag="sttw")
                nc.vector.tensor_tensor(
                    out=sttw[:], in0=rep[:],
                    in1=C["iotaP"][:, 0:1].to_broadcast([128, NE]),
                    op=mybir.AluOpType.is_equal)
                return stw, sttw

            def _post_window(layer, w, accsb, pec, rsb, denr):
                """Shared epilogue: outn -> beta gate -> h' -> transposes.
                accsb may be None (layer 0: value comes entirely from pec)."""
                outn = sb.tile([128, HID], f32, tag="outn")
                if accsb is None:
                    vsrc = pec[:]
                else:
                    vsum = sb.tile([128, HID], f32, tag="vsum")
                    nc.vector.tensor_tensor(out=vsum[:], in0=accsb[:, 0:HID],
                                            in1=pec[:], op=mybir.AluOpType.add)
                    vsrc = vsum[:]
                nc.vector.tensor_tensor(
                    out=outn[:].rearrange("p (h dd) -> p h dd", h=H),
                    in0=vsrc.rearrange("p (h dd) -> p h dd", h=H),
                    in1=denr[:].rearrange("p (h o) -> p h o", o=1).to_broadcast([128, H, D]),
                    op=mybir.AluOpType.mult)
                scrd = sb.tile([128, HID], f32, tag="scrd")
                outP = sb.tile([128, 1], f32, tag="outP")
                nc.vector.tensor_tensor(out=scrd[:], in0=outn[:],
                                        in1=C[f"prep{layer}"][:],
                                        op=mybir.AluOpType.mult)
                nc.vector.tensor_reduce(
                    out=outP[:], in_=scrd[:].rearrange("p (a b) -> p a b", a=1),
                    axis=mybir.AxisListType.XY, op=mybir.AluOpType.add)
                u = sb.tile([128, 1], f32, tag="u")
                nc.scalar.activation(out=u[:], in_=outP[:],
                                     func=mybir.ActivationFunctionType.Exp,
                                     scale=-1.0, bias=rsb[:, 160:161])
                up1 = sb.tile([128, 1], f32, tag="up1")
                nc.vector.tensor_scalar_add(up1[:], u[:], 1.0)
                beta = sb.tile([128, 1], f32, tag="beta")
                nc.vector.reciprocal(out=beta[:], in_=up1[:])
                dvec = sb.tile([128, HID], f32, tag="dvec")
                nc.vector.tensor_sub(dvec[:], rsb[:, :HID], outn[:])
                hp = sb.tile([128, HID], bf16, tag="hp")
                nc.vector.scalar_tensor_tensor(
                    out=hp[:], in0=dvec[:], scalar=beta[:, 0:1], in1=outn[:],
                    op0=mybir.AluOpType.mult, op1=mybir.AluOpType.add)
                wsl = slice(w * 128, (w + 1) * 128)
                nc.sync.dma_start(out=h_nm[layer][wsl, :], in_=hp[:])
                ptr1 = ps.tile([128, 128], bf16, tag="tp")
                nc.tensor.transpose(ptr1[:], hp[:, 0:128], C["ident"][:])
                t1 = sb.tile([128, 128], bf16, tag="t1")
                nc.scalar.copy(out=t1[:], in_=ptr1[:])
                nc.sync.dma_start(out=hT[layer][0:128, wsl], in_=t1[:])
                ptr2 = ps.tile([32, 128], bf16, tag="tp")
                nc.tensor.transpose(ptr2[:], hp[:, 128:160], C["ident"][:])
                t2 = sb.tile([32, 128], bf16, tag="t2")
                nc.scalar.copy(out=t2[:], in_=ptr2[:])
                nc.sync.dma_start(out=hT[layer][128:160, wsl], in_=t2[:])
                return t1, t2

            # ==== layer 0 edge phase (gather-free) + kv1 GEMM fold-in ====
            NW0 = 2 if phases == "mini" else WIN
            with nc.named_scope("l0"):
                for w in range(NW0):
                    wsl = slice(w * 128, (w + 1) * 128)
                    xw = sb.tile([6, 128], bf16, tag="xw", bufs=3)
                    nc.sync.dma_start(out=xw[:], in_=d["xT6"][:, wsl])
                    stw, sttw = _onehots(w)
                    # qW0 + r/beta in one PSUM bank
                    pqrb = ps.tile([128, 201], f32, tag="qrb", bufs=1)
                    nc.tensor.matmul(pqrb[:, 0:40], xw[:], C["wq0x"][:],
                                     start=True, stop=True, skip_group_check=True)
                    nc.tensor.matmul(pqrb[:, 40:201], xw[:], C["wrb0"][:],
                                     start=True, stop=True, skip_group_check=True)
                    qx = sb.tile([128, 40], bf16, tag="qx")
                    nc.scalar.copy(out=qx[:], in_=pqrb[:, 0:40])
                    rsb = sb.tile([128, 161], f32, tag="rsb")
                    nc.scalar.copy(out=rsb[:], in_=pqrb[:, 40:201])
                    # qW0 gathered to edges: NCH one-hot matmuls into one bank
                    pqg = ps.tile([128, NCH * 40], f32, tag="qg", bufs=3)
                    for j in range(NCH):
                        nc.tensor.matmul(pqg[:, j * 40 : (j + 1) * 40],
                                         sttw[:, j * 128 : (j + 1) * 128], qx[:],
                                         start=True, stop=True, skip_group_check=True)
                    # alpha = sum_c qW0_g * attr10  (batched over the window)
                    a0sl = slice(w * NCH * 40, (w + 1) * NCH * 40)
                    tqw0 = sb.tile([128, NCH * 40], bf16, tag="tqw0")
                    nc.vector.tensor_tensor(out=tqw0[:], in0=pqg[:],
                                            in1=attr0t[:, a0sl],
                                            op=mybir.AluOpType.mult)
                    alf = sb.tile([128, NCH * H], f32, tag="alf")
                    nc.vector.tensor_reduce(
                        out=alf[:], in_=tqw0[:].rearrange("p (c x) -> p c x", x=10),
                        axis=mybir.AxisListType.X, op=mybir.AluOpType.add)
                    exw = sb.tile([128, NCH * H], bf16, tag="exw")
                    nc.scalar.activation(out=exw[:], in_=alf[:],
                                         func=mybir.ActivationFunctionType.Exp,
                                         scale=INVSQD)
                    # wt0 [128, NCH, 44] = [attr10*ex (h,10) | ex (h)]
                    wt0 = sb.tile([128, NCH, 44], bf16, tag="wt0")
                    nc.vector.tensor_tensor(
                        out=wt0[:, :, 0:40].rearrange("p c (h x) -> p c h x", h=H),
                        in0=attr0t[:, a0sl].rearrange("p (c h x) -> p c h x", h=H, x=10),
                        in1=exw[:].rearrange("p (c h o) -> p c h o", h=H, o=1)
                            .to_broadcast([128, NCH, H, 10]),
                        op=mybir.AluOpType.mult)
                    nc.scalar.copy(out=wt0[:, :, 40:44],
                                   in_=exw[:].rearrange("p (c x) -> p c x", x=H))
                    pacc = ps.tile([128, 44], f32, tag="acc", bufs=1)
                    for j in range(NCH):
                        nc.tensor.matmul(pacc[:], stw[:, j * 128 : (j + 1) * 128],
                                         wt0[:, j, :], start=(j == 0),
                                         stop=(j == NCH - 1), skip_group_check=True)
                    accsb = sb.tile([128, 44], f32, tag="accsb")
                    nc.scalar.copy(out=accsb[:], in_=pacc[:])
                    dmax = sb.tile([128, H], f32, tag="dmax")
                    nc.vector.tensor_scalar_max(dmax[:], accsb[:, 40:44], 1e-30)
                    denr = sb.tile([128, H], f32, tag="denr")
                    nc.vector.reciprocal(out=denr[:], in_=dmax[:])
                    ptt = ps.tile([40, 128], f32, tag="tp")
                    nc.tensor.transpose(ptt[:], accsb[:, 0:40], C["identf"][:])
                    t40 = sb.tile([40, 128], bf16, tag="t40")
                    nc.scalar.copy(out=t40[:], in_=ptt[:])
                    pec = ps.tile([128, HID], f32, tag="tp2", bufs=1)
                    nc.tensor.matmul(pec[:], t40[:], C["wvblk0"][:],
                                     start=True, stop=True)
                    t1, t2 = _post_window(0, w, None, pec, rsb, denr)
                    # kv1 GEMM for this window (h1 just produced, transposed)
                    pkv = ps.tile([128, JK], f32, tag="qg", bufs=3)
                    nc.tensor.matmul(pkv[:], t1[:], C["wkv1a"][:], start=True, stop=False)
                    nc.tensor.matmul(pkv[:], t2[:], C["wkv1b"][:], start=False, stop=False)
                    nc.tensor.matmul(pkv[:], ones1[:, :128], C["wkv1c"][:], start=False, stop=True)
                    kvsb = sb.tile([128, JK], bf16, tag="kvsb")
                    nc.scalar.copy(out=kvsb[:], in_=pkv[:])
                    nc.sync.dma_start(out=kv_own[wsl, :], in_=kvsb[:])

            if phases in ("ag", "l1nog", "l1", "full"):
              with nc.named_scope("ag1"):
                nc.gpsimd.collective_compute(
                    "AllGather", mybir.AluOpType.bypass, replica_groups=rg,
                    ins=[kv_own[:]], outs=[kv_full[:]])

            # ==== layer 1 edge phase ====
            with nc.named_scope("l1"):
                for w in range(WIN if phases in ("l1nog", "l1", "full") else 0):
                    wsl = slice(w * 128, (w + 1) * 128)
                    # per-edge kv rows via HW-DGE indirect gathers
                    kvw = gath.tile([128, NCH, JK], bf16, tag="kvw")
                    if phases == "l1nog":
                        nc.gpsimd.memset(kvw[:], 0.0)
                    else:
                        for j in range(NCH):
                            nc.gpsimd.indirect_dma_start(
                                out=kvw[:, j, :], out_offset=None,
                                in_=kv_full[:],
                                in_offset=bass.IndirectOffsetOnAxis(
                                    ap=idxt[:, w * NCH + j : w * NCH + j + 1], axis=0),
                            )
                    hta = sb.tile([128, 128], bf16, tag="hta", bufs=3)
                    nc.sync.dma_start(out=hta[:], in_=hT[0][0:128, wsl])
                    htb = sb.tile([32, 128], bf16, tag="htb", bufs=3)
                    nc.sync.dma_start(out=htb[:], in_=hT[0][128:160, wsl])
                    stw, sttw = _onehots(w)
                    pqrb = ps.tile([128, 341], f32, tag="qrb", bufs=1)
                    nc.tensor.matmul(pqrb[:, 0:180], hta[:], C["wq1xa"][:],
                                     start=True, stop=False, skip_group_check=True)
                    nc.tensor.matmul(pqrb[:, 0:180], htb[:], C["wq1xb"][:],
                                     start=False, stop=False, skip_group_check=True)
                    nc.tensor.matmul(pqrb[:, 0:180], ones1[:, :128], C["wq1xc"][:],
                                     start=False, stop=True, skip_group_check=True)
                    nc.tensor.matmul(pqrb[:, 180:341], hta[:], C["wrb1a"][:],
                                     start=True, stop=False, skip_group_check=True)
                    nc.tensor.matmul(pqrb[:, 180:341], htb[:], C["wrb1b"][:],
                                     start=False, stop=False, skip_group_check=True)
                    nc.tensor.matmul(pqrb[:, 180:341], ones1[:, :128], C["wrb1c"][:],
                                     start=False, stop=True, skip_group_check=True)
                    qx = sb.tile([128, 180], bf16, tag="qx")
                    nc.scalar.copy(out=qx[:], in_=pqrb[:, 0:180])
                    rsb = sb.tile([128, 161], f32, tag="rsb")
                    nc.scalar.copy(out=rsb[:], in_=pqrb[:, 180:341])
                    # q gathered to edges (pairs of chunks share a PSUM bank),
                    # alpha terms into tqw
                    tqw = sb.tile([128, NCH * 180], bf16, tag="tqw")
                    e1sl = w * NCH * 20
                    for g in range((NCH + 1) // 2):
                        j0 = 2 * g
                        jn = min(2, NCH - j0)
                        pqg = ps.tile([128, 360], f32, tag="qg", bufs=3)
                        for jj in range(jn):
                            nc.tensor.matmul(
                                pqg[:, jj * 180 : (jj + 1) * 180],
                                sttw[:, (j0 + jj) * 128 : (j0 + jj + 1) * 128],
                                qx[:], start=True, stop=True, skip_group_check=True)
                        tq4 = tqw[:].rearrange("p (c h x) -> p c h x", h=H, x=45)
                        pq4 = pqg[:, 0 : jn * 180].rearrange(
                            "p (c h x) -> p c h x", h=H, x=45)
                        nc.vector.tensor_tensor(
                            out=tq4[:, j0 : j0 + jn, :, 0:40],
                            in0=pq4[:, :, :, 0:40],
                            in1=kvw[:, j0 : j0 + jn, 0:HID]
                                .rearrange("p c (h dd) -> p c h dd", h=H),
                            op=mybir.AluOpType.mult)
                        nc.vector.tensor_tensor(
                            out=tq4[:, j0 : j0 + jn, :, 40:45],
                            in0=pq4[:, :, :, 40:45],
                            in1=eaQ1t[:, e1sl + j0 * 20 : e1sl + (j0 + jn) * 20]
                                .rearrange("p (c h x) -> p c h x", h=H, x=5),
                            op=mybir.AluOpType.mult)
                    alf = sb.tile([128, NCH * H], f32, tag="alf")
                    nc.vector.tensor_reduce(
                        out=alf[:], in_=tqw[:].rearrange("p (c x) -> p c x", x=45),
                        axis=mybir.AxisListType.X, op=mybir.AluOpType.add)
                    exw = sb.tile([128, NCH * H], bf16, tag="exw")
                    nc.scalar.activation(out=exw[:], in_=alf[:],
                                         func=mybir.ActivationFunctionType.Exp,
                                         scale=INVSQD)
                    # wt [128, NCH, 184] = [v*ex (h,40) | ex (h) | ea5*ex (h,5)]
                    wt = sb.tile([128, NCH, 184], bf16, tag="wt")
                    nc.vector.tensor_tensor(
                        out=wt[:, :, 0:HID].rearrange("p c (h dd) -> p c h dd", h=H),
                        in0=kvw[:, :, HID:JK].rearrange("p c (h dd) -> p c h dd", h=H),
                        in1=exw[:].rearrange("p (c h o) -> p c h o", h=H, o=1)
                            .to_broadcast([128, NCH, H, D]),
                        op=mybir.AluOpType.mult)
                    nc.scalar.copy(out=wt[:, :, 160:164],
                                   in_=exw[:].rearrange("p (c x) -> p c x", x=H))
                    nc.vector.tensor_tensor(
                        out=wt[:, :, 164:184].rearrange("p c (h x) -> p c h x", h=H),
                        in0=eaQ1t[:, e1sl : e1sl + NCH * 20]
                            .rearrange("p (c h x) -> p c h x", h=H, x=5),
                        in1=exw[:].rearrange("p (c h o) -> p c h o", h=H, o=1)
                            .to_broadcast([128, NCH, H, 5]),
                        op=mybir.AluOpType.mult)
                    pacc = ps.tile([128, 184], f32, tag="acc", bufs=1)
                    for j in range(NCH):
                        nc.tensor.matmul(pacc[:], stw[:, j * 128 : (j + 1) * 128],
                                         wt[:, j, :], start=(j == 0),
                                         stop=(j == NCH - 1), skip_group_check=True)
                    accsb = sb.tile([128, 184], f32, tag="accsb")
                    nc.scalar.copy(out=accsb[:], in_=pacc[:])
                    dmax = sb.tile([128, H], f32, tag="dmax")
                    nc.vector.tensor_scalar_max(dmax[:], accsb[:, 160:164], 1e-30)
                    denr = sb.tile([128, H], f32, tag="denr")
                    nc.vector.reciprocal(out=denr[:], in_=dmax[:])
                    ptt = ps.tile([20, 128], f32, tag="tp")
                    nc.tensor.transpose(ptt[:], accsb[:, 164:184], C["identf"][:])
                    t20 = sb.tile([20, 128], bf16, tag="t40")
                    nc.scalar.copy(out=t20[:], in_=ptt[:])
                    pec = ps.tile([128, HID], f32, tag="tp2", bufs=1)
                    nc.tensor.matmul(pec[:], t20[:], C["weblk1"][:],
                                     start=True, stop=True)
                    _post_window(1, w, accsb, pec, rsb, denr)

            # ==== final phase: gate + graph pooling + head MLP ====
            if phases != "full":
                dummy = sb.tile([32, 6], f32, tag="osb")
                nc.gpsimd.memset(dummy[:], 0.0)
                nc.sync.dma_start(out=out_d[:], in_=dummy[:])
            with nc.named_scope("final"):
                pgr = ps.tile([32, JK + 1], f32, tag="acc", bufs=1)
                for w in range(WIN if phases == "full" else 0):
                    wsl = slice(w * 128, (w + 1) * 128)
                    h1w = sb.tile([128, HID], bf16, tag="h1w")
                    nc.sync.dma_start(out=h1w[:], in_=h_nm[0][wsl, :])
                    h2w = sb.tile([128, HID], bf16, tag="h2w")
                    nc.sync.dma_start(out=h2w[:], in_=h_nm[1][wsl, :])
                    pg = ps.tile([128, HID], f32, tag="qg", bufs=3)
                    first = True
                    for (ti, wkey) in ((0, "wg1_h1"), (1, "wg1_h2")):
                        g_a = sb.tile([128, 128], bf16, tag="hta", bufs=3)
                        nc.sync.dma_start(out=g_a[:], in_=hT[ti][0:128, wsl])
                        g_b = sb.tile([32, 128], bf16, tag="htb", bufs=3)
                        nc.sync.dma_start(out=g_b[:], in_=hT[ti][128:160, wsl])
                        nc.tensor.matmul(pg[:], g_a[:], C[wkey + "a"][:], start=first, stop=False)
                        first = False
                        nc.tensor.matmul(pg[:], g_b[:], C[wkey + "b"][:], start=False, stop=False)
                    nc.tensor.matmul(pg[:], ones1[:, :128], C["wg1_h1c"][:], start=False, stop=True)
                    grelu = sb.tile([128, HID], f32, tag="grelu")
                    nc.vector.tensor_scalar_max(grelu[:], pg[:], 0.0)
                    scr2 = sb.tile([128, HID], f32, tag="scrd")
                    gatec = sb.tile([128, 1], f32, tag="gatec")
                    nc.vector.tensor_tensor(out=scr2[:], in0=grelu[:],
                                            in1=C["wg2rep"][:],
                                            op=mybir.AluOpType.mult)
                    nc.vector.tensor_reduce(
                        out=gatec[:], in_=scr2[:].rearrange("p (a b) -> p a b", a=1),
                        axis=mybir.AxisListType.XY, op=mybir.AluOpType.add)
                    ge = sb.tile([128, 1], f32, tag="ge")
                    nc.scalar.activation(out=ge[:], in_=gatec[:],
                                         func=mybir.ActivationFunctionType.Exp,
                                         bias=C["bg2rep"][:, 0:1])
                    sg = sb.tile([128, 32], bf16, tag="sg")
                    nc.vector.tensor_tensor(
                        out=sg[:], in0=batchct[:, w : w + 1].to_broadcast([128, 32]),
                        in1=C["iota32"][:], op=mybir.AluOpType.is_equal)
                    wg = sb.tile([128, JK + 1], bf16, tag="wg")
                    nc.vector.tensor_scalar_mul(wg[:, 0:HID], h1w[:], ge[:, 0:1])
                    nc.vector.tensor_scalar_mul(wg[:, HID:JK], h2w[:], ge[:, 0:1])
                    nc.scalar.copy(out=wg[:, JK : JK + 1], in_=ge[:])
                    nc.tensor.matmul(pgr[:], sg[:], wg[:], start=(w == 0),
                                     stop=(w == WIN - 1), skip_group_check=True)
                if phases == "full":
                    pg_sb = sb.tile([32, JK + 1], f32, tag="pg_sb")
                    nc.scalar.copy(out=pg_sb[:], in_=pgr[:])
                    nc.sync.dma_start(out=pool_in[:], in_=pg_sb[:])
                    nc.gpsimd.collective_compute(
                        "AllReduce", mybir.AluOpType.add, replica_groups=rg,
                        ins=[pool_in[:]], outs=[pool_out[:]])
                    psb = sb.tile([32, JK + 1], f32, tag="psb")
                    nc.sync.dma_start(out=psb[:], in_=pool_out[:])
                    gden = sb.tile([32, 1], f32, tag="gden")
                    nc.vector.tensor_scalar_max(gden[:], psb[:, JK : JK + 1], 1e-30)
                    gdr = sb.tile([32, 1], f32, tag="gdr")
                    nc.vector.reciprocal(out=gdr[:], in_=gden[:])
                    pl = sb.tile([32, JK], bf16, tag="pl")
                    nc.vector.tensor_scalar_mul(pl[:], psb[:, 0:JK], gdr[:, 0:1])

                    def _headmm(vin, wa, wb, wc, wd, nout, tagp):
                        pouts = ps.tile([32, nout], f32, tag=tagp,
                                        bufs=(3 if tagp == "qg" else 1))
                        for si, (c0, m) in enumerate(((0, 128), (128, 128), (256, 64))):
                            ptt = ps.tile([m, 32], bf16, tag="tp")
                            nc.tensor.transpose(ptt[:], vin[:, c0 : c0 + m],
                                                C["ident"][0:32, 0:32])
                            tsb = sb.tile([m, 32], bf16, tag="tsb")
                            nc.scalar.copy(out=tsb[:], in_=ptt[:])
                            nc.tensor.matmul(pouts[:], tsb[:], (wa, wb, wc)[si][:m, :],
                                             start=(si == 0), stop=False, skip_group_check=True)
                        nc.tensor.matmul(pouts[:], ones1[:, :32], wd[:],
                                         start=False, stop=True, skip_group_check=True)
                        return pouts

                    ph1 = _headmm(pl, C["wh1a"], C["wh1b"], C["wh1c"], C["wh1d"], JK, "qrb")
                    vrel = sb.tile([32, JK], bf16, tag="vrel")
                    nc.vector.tensor_scalar_max(vrel[:], ph1[:], 0.0)
                    ph2 = _headmm(vrel, C["wh2a"], C["wh2b"], C["wh2c"], C["wh2d"], 6, "qg")
                    osb = sb.tile([32, 6], f32, tag="osb")
                    nc.scalar.copy(out=osb[:], in_=ph2[:])
                    nc.sync.dma_start(out=out_d[:], in_=osb[:])

    nc.compile()
    return nc


_CACHE = {}
_LAST_RES = None


def kernel(**inputs):
    inputs = {k: np.asarray(v) for k, v in inputs.items()}
    per_core, NCH = _preprocess(
        inputs["x"], inputs["edge_index"], inputs["edge_attr"], inputs["batch"])
    w = _weights(inputs)
    import os as _os
    phases = _os.environ.get("KERNEL_PHASES", "full")
    key = (NCH, phases)
    if key not in _CACHE:
        _CACHE[key] = _build(NCH, phases)
    nc = _CACHE[key]
    in_maps = []
    for r in range(NCORES):
        m = dict(w)
        m.update(per_core[r])
        in_maps.append(m)
    import os
    trace = bool(os.environ.get("KERNEL_TRACE"))
    if trace:
        try:
            import axon_prof
            axon_prof.install()
        except Exception:
            trace = False
    res = run_bass_kernel_spmd(nc, in_maps, core_ids=list(range(NCORES)), trace=trace)
    if trace and res.exec_time_ns is not None:
        print(f"HW exec time: {res.exec_time_ns} ns")
        if res.per_core_scope_times:
            for scope, cores in sorted(res.per_core_scope_times.items()):
                print(f"  scope {scope}: {cores}")
    global _LAST_RES
    _LAST_RES = res
    out = res.results[0]["out"]
    return out.reshape(G, 2, 3).astype(np.float32)


# revision 13
# speedup vs baseline: 3.4409x; 1.2477x over previous
"""Trainium2 Bass kernel for nn_EndpointRegressor (2x TransformerConv GNN +
AttentionalAggregation) distributed over 8 NeuronCores.

Sharding: edges partitioned by destination node range (6272 nodes/core);
each core owns its dst nodes exclusively, so segment softmax/scatter stats
need no cross-core reduction.

Layer 0 is gather-free: k0/v0 are low-rank in host-known inputs
(k0[src] = x[src]@(W_in Wk) + ea@We + bias), so alpha0 = sum_c attr10[e,c] *
qW0[dst,h,c] with attr10 = [x[src](5), ea(4), 1] riding the edge stream, and
the value scatter accumulates T[dst,h,c] = sum_e ex*attr10 which is expanded
to 160 dims by one small matmul per 128-node window.  No kv GEMM, no
AllGather, no gather for layer 0.

Layer 1 computes kv for own nodes, AllGathers the [50176,320] bf16 table,
and gathers per-edge rows with indirect_dma_start (int32 indices, HW DGE).
The edge-feature term is folded the same way (qWe trick) so no per-chunk
e-matmul.  Per 128-edge chunk only two PE matmuls remain (q-gather via
one-hot, scatter via one-hot); all per-edge vector math is batched per
128-dst-node window.  Segment softmax uses exp without max subtraction
(alpha ~ +-0.1 for this model family); division by the denominator happens
on the node side.  All matmul operands bf16, PSUM accumulation fp32.

The window loops are software-pipelined (front half of window w+1 emitted
before the back half of window w) so the PE/DVE/Act ping-pong latency of one
window overlaps the next window's independent work.  The node->edge one-hot
(sttw) is host-precomputed and DMAed; the edge->node one-hot (stw) is built
on DVE from bf16 operands.  The only scalar-engine activation functions are
Exp and Copy (one table set, no ACT_TABLE_LOAD thrash); sigmoid is computed
as 1/(1+exp(-z)) with the gate bias pre-negated on the host.
"""
import math
import numpy as np
import ml_dtypes

import concourse.bass as bass
import concourse.bacc as bacc
import concourse.mybir as mybir
import concourse.tile as tile
from concourse._compat import get_trn_type
from concourse.bass_utils import run_bass_kernel_spmd
from concourse.library_config import mlp

# ---- problem constants (fixed by the problem spec) ----
N, E, G = 50000, 500000, 32
H, D = 4, 40
HID = H * D            # 160
JK = 2 * HID           # 320
NCORES = 8
NSHARD = 6272          # 49*128 nodes per core
NPAD = NCORES * NSHARD # 50176
WIN = NSHARD // 128    # 49
INVSQD = 1.0 / math.sqrt(float(D))

f32 = mybir.dt.float32
bf16 = mybir.dt.bfloat16
i32 = mybir.dt.int32
nbf = ml_dtypes.bfloat16


def _preprocess(x, edge_index, edge_attr, batch):
    """Sort edges by dst, shard by dst range, pad each (core,window) bucket
    to NCH chunks of 128 edge slots. Build the per-edge attribute streams."""
    src = np.asarray(edge_index[0], dtype=np.int64)
    dst = np.asarray(edge_index[1], dtype=np.int64)
    ea = np.asarray(edge_attr, dtype=np.float32)
    x = np.asarray(x, np.float32)
    order = np.argsort(dst, kind="stable")
    src, dst, ea = src[order], dst[order], ea[order]

    core = dst // NSHARD
    win = (dst % NSHARD) // 128

    buckets = {}
    for r in range(NCORES):
        m_r = core == r
        for w in range(WIN):
            buckets[(r, w)] = np.nonzero(m_r & (win == w))[0]

    NCH = max(1, max((len(b) + 127) // 128 for b in buckets.values()))

    per_core = []
    for r in range(NCORES):
        idxE = np.zeros((128, WIN * NCH), np.int32)
        attr0 = np.zeros((128, WIN * NCH * 40), np.float32)
        eaQ1 = np.zeros((128, WIN * NCH * 20), np.float32)
        drwC = np.full((128, WIN * NCH), -1.0, np.float32)
        drwR = np.full((WIN, NCH * 128), -1.0, np.float32)
        for w in range(WIN):
            eidx = buckets[(r, w)]
            n = len(eidx)
            kk = np.arange(n)
            jj = kk // 128
            pp = kk % 128
            col = w * NCH + jj
            idxE[pp, col] = src[eidx]
            dd = (dst[eidx] % 128).astype(np.float32)
            drwC[pp, col] = dd
            drwR[w, jj * 128 + pp] = dd
            # attr10 = [x[src](5), ea(4), 1] replicated per head
            a10 = np.concatenate(
                [x[src[eidx]], ea[eidx], np.ones((n, 1), np.float32)], 1)  # [n,10]
            ea5 = np.concatenate([ea[eidx], np.ones((n, 1), np.float32)], 1)  # [n,5]
            for h in range(H):
                c0 = col * 40 + h * 10
                for c in range(10):
                    attr0[pp, c0 + c] = a10[:, c]
                c1 = col * 20 + h * 5
                for c in range(5):
                    eaQ1[pp, c1 + c] = ea5[:, c]
        n0 = r * NSHARD
        xT6 = np.zeros((6, NSHARD), np.float32)
        xT6[5, :] = 1.0
        batchc = np.full((128, WIN), -1.0, np.float32)
        n_real = max(0, min(NSHARD, N - n0))
        if n_real > 0:
            xT6[:5, :n_real] = x[n0 : n0 + n_real].T
            bfull = np.full(NSHARD, -1.0, np.float32)
            bfull[:n_real] = np.asarray(batch[n0 : n0 + n_real], np.float32)
            batchc[:, :] = bfull.reshape(WIN, 128).T
        # host-built one-hot: sttw[w, p, e] = (dst%128 of edge e == p)
        sttw = (drwR[:, None, :] == np.arange(128, dtype=np.float32)[None, :, None])
        per_core.append(
            dict(
                xT6=xT6.astype(nbf),
                idxE=idxE,
                attr0=attr0.astype(nbf),
                eaQ1=eaQ1.astype(nbf),
                drwC=drwC.astype(nbf),
                sttw=sttw.astype(nbf),
                batchc=batchc.astype(nbf),
            )
        )
    return per_core, NCH


def _weights(inp):
    """Host-side weight packing/folding (fp64 math, bf16 output)."""
    w = {}
    f8 = np.float64
    W_in = inp["W_in"].astype(f8)
    b_in = inp["b_in"].astype(f8)

    w["iota128"] = np.broadcast_to(np.arange(128, dtype=np.float32), (128, 128)).astype(nbf).copy()
    w["iota32"] = np.broadcast_to(np.arange(32, dtype=np.float32), (128, 32)).astype(nbf).copy()
    w["ident"] = np.eye(128, dtype=np.float32).astype(nbf)
    w["identf"] = np.eye(128, dtype=np.float32)
    Wg1 = inp["Wg1"].astype(np.float32)
    w["wg1_h1"] = np.concatenate([Wg1[:HID], inp["bg1"].astype(np.float32)[None, :]], 0).astype(nbf)
    w["wg1_h2"] = np.concatenate([Wg1[HID:], np.zeros((1, HID), np.float32)], 0).astype(nbf)
    w["wg2rep"] = np.broadcast_to(inp["Wg2"].astype(np.float32)[:, 0], (128, HID)).copy()
    w["wh1"] = np.concatenate([inp["Wh1"].astype(np.float32), inp["bh1"].astype(np.float32)[None, :]], 0).astype(nbf)
    w["wh2"] = np.concatenate([inp["Wh2"].astype(np.float32), inp["bh2"].astype(np.float32)[None, :]], 0).astype(nbf)
    w["bg2rep"] = np.full((128, 1), float(np.asarray(inp["bg2"]).reshape(-1)[0]), np.float32)

    for layer in range(2):
        Wq, Wk, Wv = (inp[k][layer].astype(f8) for k in ("Wq", "Wk", "Wv"))
        bq, bk, bv = (inp[k][layer].astype(f8) for k in ("bq", "bk", "bv"))
        Wskip, bskip = inp["Wskip"][layer].astype(f8), inp["bskip"][layer].astype(f8)
        Wbeta = inp["Wbeta"][layer].astype(f8)
        We, be = inp["We"][layer].astype(f8), inp["be"][layer].astype(f8)
        P = (Wbeta[:HID, 0] + Wbeta[2 * HID :, 0])
        Q = (Wbeta[HID : 2 * HID, 0] - Wbeta[2 * HID :, 0])
        w[f"prep{layer}"] = np.broadcast_to(P.astype(np.float32), (128, HID)).copy()
        if layer == 0:
            Q6 = np.concatenate([W_in @ Wq, (b_in @ Wq + bq)[None, :]], 0)     # [6,160]
            K10 = np.concatenate([W_in @ Wk, We, (b_in @ Wk + bk + be)[None, :]], 0)  # [10,160]
            V10 = np.concatenate([W_in @ Wv, We, (b_in @ Wv + bv + be)[None, :]], 0)  # [10,160]
            wq0x = np.zeros((6, 40), f8)
            wvblk0 = np.zeros((40, HID), f8)
            for h in range(H):
                ds = slice(h * D, (h + 1) * D)
                wq0x[:, h * 10 : (h + 1) * 10] = np.einsum(
                    "fd,cd->fc", Q6[:, ds], K10[:, ds])
                wvblk0[h * 10 : (h + 1) * 10, ds] = V10[:, ds]
            S6 = np.concatenate([W_in @ Wskip, (b_in @ Wskip + bskip)[None, :]], 0)  # [6,160]
            wrb0 = np.concatenate([S6, -(S6 @ Q)[:, None]], 1)            # [6,161]
            w["wq0x"] = wq0x.astype(np.float32).astype(nbf)
            w["wvblk0"] = wvblk0.astype(np.float32).astype(nbf)
            w["wrb0"] = wrb0.astype(np.float32).astype(nbf)
        else:
            We5 = np.concatenate([We, be[None, :]], 0)                    # [5,160]
            Q161 = np.concatenate([Wq, bq[None, :]], 0)                   # [161,160]
            wq1x = np.zeros((161, 180), f8)
            weblk1 = np.zeros((20, HID), f8)
            for h in range(H):
                ds = slice(h * D, (h + 1) * D)
                wq1x[:, h * 45 : h * 45 + 40] = Q161[:, ds]
                wq1x[:, h * 45 + 40 : h * 45 + 45] = np.einsum(
                    "fd,cd->fc", Q161[:, ds], We5[:, ds])
                weblk1[h * 5 : (h + 1) * 5, ds] = We5[:, ds]
            S161 = np.concatenate([Wskip, bskip[None, :]], 0)             # [161,160]
            wrb1 = np.concatenate([S161, -(S161 @ Q)[:, None]], 1)        # [161,161]
            wkv1 = np.concatenate(
                [np.concatenate([Wk, Wv], 1), np.concatenate([bk, bv])[None, :]], 0)  # [161,320]
            w["wq1x"] = wq1x.astype(np.float32).astype(nbf)
            w["weblk1"] = weblk1.astype(np.float32).astype(nbf)
            w["wrb1"] = wrb1.astype(np.float32).astype(nbf)
            w["wkv1"] = wkv1.astype(np.float32).astype(nbf)
    return w


def _build(NCH, phases="full"):
    nc = bacc.Bacc(get_trn_type() or "TRN2", target_bir_lowering=False)
    NE = NCH * 128

    # ---- dram I/O ----
    d = {}
    d["xT6"] = nc.dram_tensor("xT6", [6, NSHARD], bf16, kind="ExternalInput")
    d["idxE"] = nc.dram_tensor("idxE", [128, WIN * NCH], i32, kind="ExternalInput")
    d["attr0"] = nc.dram_tensor("attr0", [128, WIN * NCH * 40], bf16, kind="ExternalInput")
    d["eaQ1"] = nc.dram_tensor("eaQ1", [128, WIN * NCH * 20], bf16, kind="ExternalInput")
    d["drwC"] = nc.dram_tensor("drwC", [128, WIN * NCH], bf16, kind="ExternalInput")
    d["sttw"] = nc.dram_tensor("sttw", [WIN, 128, NE], bf16, kind="ExternalInput")
    d["batchc"] = nc.dram_tensor("batchc", [128, WIN], bf16, kind="ExternalInput")
    wshapes = dict(
        iota128=([128, 128], bf16), iota32=([128, 32], bf16),
        ident=([128, 128], bf16), identf=([128, 128], f32),
        wg1_h1=([161, HID], bf16), wg1_h2=([161, HID], bf16), wg2rep=([128, HID], f32),
        wh1=([321, JK], bf16), wh2=([321, 6], bf16), bg2rep=([128, 1], f32),
        prep0=([128, HID], f32), prep1=([128, HID], f32),
        wq0x=([6, 40], bf16), wvblk0=([40, HID], bf16), wrb0=([6, 161], bf16),
        wq1x=([161, 180], bf16), weblk1=([20, HID], bf16), wrb1=([161, 161], bf16),
        wkv1=([161, JK], bf16),
    )
    for k, (shp, dt_) in wshapes.items():
        d[k] = nc.dram_tensor(k, shp, dt_, kind="ExternalInput")
    out_d = nc.dram_tensor("out", [32, 6], f32, kind="ExternalOutput")

    hT = [nc.dram_tensor(f"hT{i}", [HID, NSHARD], bf16) for i in (1, 2)]
    h_nm = [nc.dram_tensor(f"h_nm{i}", [NSHARD, HID], bf16) for i in (1, 2)]
    kv_own = nc.dram_tensor("kv_own1", [NSHARD, JK], bf16)
    kv_full = nc.dram_tensor("kv_full1", [NPAD, JK], bf16, addr_space="Shared")
    pool_in = nc.dram_tensor("pool_in", [32, JK + 1], f32)
    pool_out = nc.dram_tensor("pool_out", [32, JK + 1], f32, addr_space="Shared")
    rg = [list(range(NCORES))]

    with tile.TileContext(nc) as tc:
        with (
            tc.tile_pool(name="cst", bufs=1) as cst,
            tc.tile_pool(name="sb", bufs=2) as sb,
            tc.tile_pool(name="gath", bufs=3) as gath,
            tc.tile_pool(name="ps", bufs=2, space="PSUM") as ps,
        ):
            nc.gpsimd.load_library(mlp)

            # ---- persistent constants + preloads ----
            C = {}
            def _load_const(key, part, cols, row0=0):
                t = cst.tile([part, cols], wshapes[key][1], name=f"c_{key}_{row0}")
                nc.sync.dma_start(out=t[:], in_=d[key][row0 : row0 + part, :])
                return t
            for key in ("iota128", "iota32", "ident", "identf", "wg2rep",
                        "bg2rep", "prep0", "prep1", "wvblk0", "weblk1"):
                C[key] = _load_const(key, wshapes[key][0][0], wshapes[key][0][1])
            C["wq0x"] = _load_const("wq0x", 6, 40)
            C["wrb0"] = _load_const("wrb0", 6, 161)
            for key, cols in (("wq1x", 180), ("wrb1", 161), ("wkv1", JK),
                              ("wg1_h1", HID), ("wg1_h2", HID)):
                C[key + "a"] = _load_const(key, 128, cols)
                C[key + "b"] = _load_const(key, 32, cols, 128)
                C[key + "c"] = _load_const(key, 1, cols, 160)
            C["wh1a"] = _load_const("wh1", 128, JK)
            C["wh1b"] = _load_const("wh1", 128, JK, 128)
            C["wh1c"] = _load_const("wh1", 64, JK, 256)
            C["wh1d"] = _load_const("wh1", 1, JK, 320)
            C["wh2a"] = _load_const("wh2", 128, 6)
            C["wh2b"] = _load_const("wh2", 128, 6, 128)
            C["wh2c"] = _load_const("wh2", 64, 6, 256)
            C["wh2d"] = _load_const("wh2", 1, 6, 320)

            idxt = cst.tile([128, WIN * NCH], i32, name="idxt")
            nc.sync.dma_start(out=idxt[:], in_=d["idxE"][:])
            attr0t = cst.tile([128, WIN * NCH * 40], bf16, name="attr0t")
            nc.sync.dma_start(out=attr0t[:], in_=d["attr0"][:])
            eaQ1t = cst.tile([128, WIN * NCH * 20], bf16, name="eaQ1t")
            nc.sync.dma_start(out=eaQ1t[:], in_=d["eaQ1"][:])
            drwCt = cst.tile([128, WIN * NCH], bf16, name="drwCt")
            nc.sync.dma_start(out=drwCt[:], in_=d["drwC"][:])
            batchct = cst.tile([128, WIN], bf16, name="batchct")
            nc.sync.dma_start(out=batchct[:], in_=d["batchc"][:])
            xt6t = cst.tile([6, NSHARD], bf16, name="xt6t")
            nc.sync.dma_start(out=xt6t[:], in_=d["xT6"][:])

            ones1 = cst.tile([1, 128], bf16, name="ones1")
            nc.gpsimd.memset(ones1[:], 1.0)

            def _masks(w):
                """stw (edge p -> node one-hot) on DVE; sttw host-DMAed."""
                stw = sb.tile([128, NE], bf16, tag="stw")
                nc.vector.tensor_tensor(
                    out=stw[:].rearrange("p (c x) -> p c x", x=128),
                    in0=drwCt[:, w * NCH : (w + 1) * NCH]
                        .rearrange("p (c o) -> p c o", o=1)
                        .to_broadcast([128, NCH, 128]),
                    in1=C["iota128"][:].rearrange("p (o x) -> p o x", o=1)
                        .to_broadcast([128, NCH, 128]),
                    op=mybir.AluOpType.is_equal)
                sttw = sb.tile([128, NE], bf16, tag="sttw")
                nc.sync.dma_start(out=sttw[:], in_=d["sttw"][w])
                return stw, sttw

            def _post_tail(layer, w, vsrc, rsb, denr):
                """outn -> beta gate -> h' -> transposes; returns (t1, t2)."""
                outn = sb.tile([128, HID], f32, tag="outn")
                nc.vector.tensor_tensor(
                    out=outn[:].rearrange("p (h dd) -> p h dd", h=H),
                    in0=vsrc.rearrange("p (h dd) -> p h dd", h=H),
                    in1=denr[:].rearrange("p (h o) -> p h o", o=1).to_broadcast([128, H, D]),
                    op=mybir.AluOpType.mult)
                scrd = sb.tile([128, HID], f32, tag="scrd")
                outP = sb.tile([128, 1], f32, tag="outP")
                nc.vector.tensor_tensor(out=scrd[:], in0=outn[:],
                                        in1=C[f"prep{layer}"][:],
                                        op=mybir.AluOpType.mult)
                nc.vector.tensor_reduce(
                    out=outP[:], in_=scrd[:].rearrange("p (a b) -> p a b", a=1),
                    axis=mybir.AxisListType.XY, op=mybir.AluOpType.add)
                u = sb.tile([128, 1], f32, tag="u")
                nc.scalar.activation(out=u[:], in_=outP[:],
                                     func=mybir.ActivationFunctionType.Exp,
                                     scale=-1.0, bias=rsb[:, 160:161])
                up1 = sb.tile([128, 1], f32, tag="up1")
                nc.vector.tensor_scalar_add(up1[:], u[:], 1.0)
                beta = sb.tile([128, 1], f32, tag="beta")
                nc.vector.reciprocal(out=beta[:], in_=up1[:])
                dvec = sb.tile([128, HID], f32, tag="dvec")
                nc.vector.tensor_sub(dvec[:], rsb[:, :HID], outn[:])
                hp = sb.tile([128, HID], bf16, tag="hp")
                nc.vector.scalar_tensor_tensor(
                    out=hp[:], in0=dvec[:], scalar=beta[:, 0:1], in1=outn[:],
                    op0=mybir.AluOpType.mult, op1=mybir.AluOpType.add)
                wsl = slice(w * 128, (w + 1) * 128)
                nc.sync.dma_start(out=h_nm[layer][wsl, :], in_=hp[:])
                ptr1 = ps.tile([128, 128], bf16, tag="tp")
                nc.tensor.transpose(ptr1[:], hp[:, 0:128], C["ident"][:])
                t1 = sb.tile([128, 128], bf16, tag="t1")
                nc.scalar.copy(out=t1[:], in_=ptr1[:])
                nc.sync.dma_start(out=hT[layer][0:128, wsl], in_=t1[:])
                ptr2 = ps.tile([32, 128], bf16, tag="tp")
                nc.tensor.transpose(ptr2[:], hp[:, 128:160], C["ident"][:])
                t2 = sb.tile([32, 128], bf16, tag="t2")
                nc.scalar.copy(out=t2[:], in_=ptr2[:])
                nc.sync.dma_start(out=hT[layer][128:160, wsl], in_=t2[:])
                return t1, t2

            # ==== layer 0 (gather-free) + kv1 GEMM fold-in, pipelined ====
            def l0_front(w):
                wsl = slice(w * 128, (w + 1) * 128)
                stw, sttw = _masks(w)
                pqrb = ps.tile([128, 201], f32, tag="qrb", bufs=2)
                nc.tensor.matmul(pqrb[:, 0:40], xt6t[:, wsl], C["wq0x"][:],
                                 start=True, stop=True, skip_group_check=True)
                nc.tensor.matmul(pqrb[:, 40:201], xt6t[:, wsl], C["wrb0"][:],
                                 start=True, stop=True, skip_group_check=True)
                qx = sb.tile([128, 40], bf16, tag="qx")
                nc.scalar.copy(out=qx[:], in_=pqrb[:, 0:40])
                rsb = sb.tile([128, 161], f32, tag="rsb")
                nc.scalar.copy(out=rsb[:], in_=pqrb[:, 40:201])
                pqg = ps.tile([128, NCH * 40], f32, tag="qg", bufs=3)
                for j in range(NCH):
                    nc.tensor.matmul(pqg[:, j * 40 : (j + 1) * 40],
                                     sttw[:, j * 128 : (j + 1) * 128], qx[:],
                                     start=True, stop=True, skip_group_check=True)
                a0sl = slice(w * NCH * 40, (w + 1) * NCH * 40)
                tqw0 = sb.tile([128, NCH * 40], bf16, tag="tqw0")
                nc.vector.tensor_tensor(out=tqw0[:], in0=pqg[:],
                                        in1=attr0t[:, a0sl],
                                        op=mybir.AluOpType.mult)
                alf = sb.tile([128, NCH * H], f32, tag="alf")
                nc.vector.tensor_reduce(
                    out=alf[:], in_=tqw0[:].rearrange("p (c x) -> p c x", x=10),
                    axis=mybir.AxisListType.X, op=mybir.AluOpType.add)
                exw = sb.tile([128, NCH * H], bf16, tag="exw")
                nc.scalar.activation(out=exw[:], in_=alf[:],
                                     func=mybir.ActivationFunctionType.Exp,
                                     scale=INVSQD)
                wt0 = sb.tile([128, NCH, 44], bf16, tag="wt0")
                nc.vector.tensor_tensor(
                    out=wt0[:, :, 0:40].rearrange("p c (h x) -> p c h x", h=H),
                    in0=attr0t[:, a0sl].rearrange("p (c h x) -> p c h x", h=H, x=10),
                    in1=exw[:].rearrange("p (c h o) -> p c h o", h=H, o=1)
                        .to_broadcast([128, NCH, H, 10]),
                    op=mybir.AluOpType.mult)
                nc.scalar.copy(out=wt0[:, :, 40:44],
                               in_=exw[:].rearrange("p (c x) -> p c x", x=H))
                return dict(stw=stw, wt0=wt0, rsb=rsb)

            def l0_back(w, st):
                wsl = slice(w * 128, (w + 1) * 128)
                pacc = ps.tile([128, 204], f32, tag="acc", bufs=1)
                for j in range(NCH):
                    nc.tensor.matmul(pacc[:, 0:44],
                                     st["stw"][:, j * 128 : (j + 1) * 128],
                                     st["wt0"][:, j, :], start=(j == 0),
                                     stop=(j == NCH - 1), skip_group_check=True)
                accsb = sb.tile([128, 44], f32, tag="accsb")
                nc.scalar.copy(out=accsb[:], in_=pacc[:, 0:44])
                dmax = sb.tile([128, H], f32, tag="dmax")
                nc.vector.tensor_scalar_max(dmax[:], accsb[:, 40:44], 1e-30)
                denr = sb.tile([128, H], f32, tag="denr")
                nc.vector.reciprocal(out=denr[:], in_=dmax[:])
                ptt = ps.tile([40, 128], f32, tag="tp")
                nc.tensor.transpose(ptt[:], accsb[:, 0:40], C["identf"][:])
                t40 = sb.tile([40, 128], bf16, tag="t40")
                nc.scalar.copy(out=t40[:], in_=ptt[:])
                nc.tensor.matmul(pacc[:, 44:204], t40[:], C["wvblk0"][:],
                                 start=True, stop=True, skip_group_check=True)
                t1, t2 = _post_tail(0, w, pacc[:, 44:204], st["rsb"], denr)
                pkv = ps.tile([128, JK], f32, tag="qg", bufs=3)
                nc.tensor.matmul(pkv[:], t1[:], C["wkv1a"][:], start=True, stop=False)
                nc.tensor.matmul(pkv[:], t2[:], C["wkv1b"][:], start=False, stop=False)
                nc.tensor.matmul(pkv[:], ones1[:, :128], C["wkv1c"][:], start=False, stop=True)
                kvsb = sb.tile([128, JK], bf16, tag="kvsb")
                nc.scalar.copy(out=kvsb[:], in_=pkv[:])
                nc.sync.dma_start(out=kv_own[wsl, :], in_=kvsb[:])

            NW0 = 2 if phases == "mini" else WIN
            with nc.named_scope("l0"):
                stp = l0_front(0)
                for w in range(1, NW0):
                    stn = l0_front(w)
                    l0_back(w - 1, stp)
                    stp = stn
                l0_back(NW0 - 1, stp)

            if phases in ("ag", "l1nog", "l1", "full"):
                with nc.named_scope("ag1"):
                    nc.gpsimd.collective_compute(
                        "AllGather", mybir.AluOpType.bypass, replica_groups=rg,
                        ins=[kv_own[:]], outs=[kv_full[:]])

            # ==== layer 1, pipelined ====
            def l1_front(w):
                wsl = slice(w * 128, (w + 1) * 128)
                kvw = gath.tile([128, NCH, JK], bf16, tag="kvw")
                if phases == "l1nog":
                    nc.gpsimd.memset(kvw[:], 0.0)
                else:
                    for j in range(NCH):
                        nc.gpsimd.indirect_dma_start(
                            out=kvw[:, j, :], out_offset=None,
                            in_=kv_full[:],
                            in_offset=bass.IndirectOffsetOnAxis(
                                ap=idxt[:, w * NCH + j : w * NCH + j + 1], axis=0),
                        )
                hta = sb.tile([128, 128], bf16, tag="hta", bufs=3)
                nc.sync.dma_start(out=hta[:], in_=hT[0][0:128, wsl])
                htb = sb.tile([32, 128], bf16, tag="htb", bufs=3)
                nc.sync.dma_start(out=htb[:], in_=hT[0][128:160, wsl])
                stw, sttw = _masks(w)
                pqrb = ps.tile([128, 341], f32, tag="qrb", bufs=2)
                nc.tensor.matmul(pqrb[:, 0:180], hta[:], C["wq1xa"][:],
                                 start=True, stop=False, skip_group_check=True)
                nc.tensor.matmul(pqrb[:, 0:180], htb[:], C["wq1xb"][:],
                                 start=False, stop=False, skip_group_check=True)
                nc.tensor.matmul(pqrb[:, 0:180], ones1[:, :128], C["wq1xc"][:],
                                 start=False, stop=True, skip_group_check=True)
                nc.tensor.matmul(pqrb[:, 180:341], hta[:], C["wrb1a"][:],
                                 start=True, stop=False, skip_group_check=True)
                nc.tensor.matmul(pqrb[:, 180:341], htb[:], C["wrb1b"][:],
                                 start=False, stop=False, skip_group_check=True)
                nc.tensor.matmul(pqrb[:, 180:341], ones1[:, :128], C["wrb1c"][:],
                                 start=False, stop=True, skip_group_check=True)
                qx = sb.tile([128, 180], bf16, tag="qx")
                nc.scalar.copy(out=qx[:], in_=pqrb[:, 0:180])
                rsb = sb.tile([128, 161], f32, tag="rsb")
                nc.scalar.copy(out=rsb[:], in_=pqrb[:, 180:341])
                tqw = sb.tile([128, NCH * 180], bf16, tag="tqw")
                e1sl = w * NCH * 20
                for g in range((NCH + 1) // 2):
                    j0 = 2 * g
                    jn = min(2, NCH - j0)
                    pqg = ps.tile([128, 360], f32, tag="qg", bufs=3)
                    for jj in range(jn):
                        nc.tensor.matmul(
                            pqg[:, jj * 180 : (jj + 1) * 180],
                            sttw[:, (j0 + jj) * 128 : (j0 + jj + 1) * 128],
                            qx[:], start=True, stop=True, skip_group_check=True)
                    tq4 = tqw[:].rearrange("p (c h x) -> p c h x", h=H, x=45)
                    pq4 = pqg[:, 0 : jn * 180].rearrange(
                        "p (c h x) -> p c h x", h=H, x=45)
                    nc.vector.tensor_tensor(
                        out=tq4[:, j0 : j0 + jn, :, 0:40],
                        in0=pq4[:, :, :, 0:40],
                        in1=kvw[:, j0 : j0 + jn, 0:HID]
                            .rearrange("p c (h dd) -> p c h dd", h=H),
                        op=mybir.AluOpType.mult)
                    nc.vector.tensor_tensor(
                        out=tq4[:, j0 : j0 + jn, :, 40:45],
                        in0=pq4[:, :, :, 40:45],
                        in1=eaQ1t[:, e1sl + j0 * 20 : e1sl + (j0 + jn) * 20]
                            .rearrange("p (c h x) -> p c h x", h=H, x=5),
                        op=mybir.AluOpType.mult)
                alf = sb.tile([128, NCH * H], f32, tag="alf")
                nc.vector.tensor_reduce(
                    out=alf[:], in_=tqw[:].rearrange("p (c x) -> p c x", x=45),
                    axis=mybir.AxisListType.X, op=mybir.AluOpType.add)
                exw = sb.tile([128, NCH * H], bf16, tag="exw")
                nc.scalar.activation(out=exw[:], in_=alf[:],
                                     func=mybir.ActivationFunctionType.Exp,
                                     scale=INVSQD)
                wt = sb.tile([128, NCH, 184], bf16, tag="wt")
                nc.vector.tensor_tensor(
                    out=wt[:, :, 0:HID].rearrange("p c (h dd) -> p c h dd", h=H),
                    in0=kvw[:, :, HID:JK].rearrange("p c (h dd) -> p c h dd", h=H),
                    in1=exw[:].rearrange("p (c h o) -> p c h o", h=H, o=1)
                        .to_broadcast([128, NCH, H, D]),
                    op=mybir.AluOpType.mult)
                nc.scalar.copy(out=wt[:, :, 160:164],
                               in_=exw[:].rearrange("p (c x) -> p c x", x=H))
                nc.vector.tensor_tensor(
                    out=wt[:, :, 164:184].rearrange("p c (h x) -> p c h x", h=H),
                    in0=eaQ1t[:, e1sl : e1sl + NCH * 20]
                        .rearrange("p (c h x) -> p c h x", h=H, x=5),
                    in1=exw[:].rearrange("p (c h o) -> p c h o", h=H, o=1)
                        .to_broadcast([128, NCH, H, 5]),
                    op=mybir.AluOpType.mult)
                return dict(stw=stw, wt=wt, rsb=rsb)

            def l1_back(w, st):
                pacc = ps.tile([128, 184], f32, tag="acc", bufs=1)
                for j in range(NCH):
                    nc.tensor.matmul(pacc[:], st["stw"][:, j * 128 : (j + 1) * 128],
                                     st["wt"][:, j, :], start=(j == 0),
                                     stop=(j == NCH - 1), skip_group_check=True)
                accsb = sb.tile([128, 184], f32, tag="accsb")
                nc.scalar.copy(out=accsb[:], in_=pacc[:])
                dmax = sb.tile([128, H], f32, tag="dmax")
                nc.vector.tensor_scalar_max(dmax[:], accsb[:, 160:164], 1e-30)
                denr = sb.tile([128, H], f32, tag="denr")
                nc.vector.reciprocal(out=denr[:], in_=dmax[:])
                ptt = ps.tile([20, 128], f32, tag="tp")
                nc.tensor.transpose(ptt[:], accsb[:, 164:184], C["identf"][:])
                t20 = sb.tile([20, 128], bf16, tag="t40")
                nc.scalar.copy(out=t20[:], in_=ptt[:])
                # e-contribution accumulates straight onto the v-sums in PSUM
                nc.tensor.matmul(pacc[:, 0:HID], t20[:], C["weblk1"][:],
                                 start=False, stop=True, skip_group_check=True)
                _post_tail(1, w, pacc[:, 0:HID], st["rsb"], denr)

            with nc.named_scope("l1"):
                if phases in ("l1nog", "l1", "full"):
                    stp = l1_front(0)
                    for w in range(1, WIN):
                        stn = l1_front(w)
                        l1_back(w - 1, stp)
                        stp = stn
                    l1_back(WIN - 1, stp)

            # ==== final phase: gate + graph pooling + head MLP ====
            if phases != "full":
                dummy = sb.tile([32, 6], f32, tag="osb")
                nc.gpsimd.memset(dummy[:], 0.0)
                nc.sync.dma_start(out=out_d[:], in_=dummy[:])
            with nc.named_scope("final"):
                pgr = ps.tile([32, JK + 1], f32, tag="acc", bufs=1)

                def fin_front(w):
                    wsl = slice(w * 128, (w + 1) * 128)
                    h1w = sb.tile([128, HID], bf16, tag="h1w")
                    nc.sync.dma_start(out=h1w[:], in_=h_nm[0][wsl, :])
                    h2w = sb.tile([128, HID], bf16, tag="h2w")
                    nc.sync.dma_start(out=h2w[:], in_=h_nm[1][wsl, :])
                    pg = ps.tile([128, HID], f32, tag="qg", bufs=3)
                    first = True
                    for (ti, wkey) in ((0, "wg1_h1"), (1, "wg1_h2")):
                        g_a = sb.tile([128, 128], bf16, tag="hta", bufs=3)
                        nc.sync.dma_start(out=g_a[:], in_=hT[ti][0:128, wsl])
                        g_b = sb.tile([32, 128], bf16, tag="htb", bufs=3)
                        nc.sync.dma_start(out=g_b[:], in_=hT[ti][128:160, wsl])
                        nc.tensor.matmul(pg[:], g_a[:], C[wkey + "a"][:], start=first, stop=False)
                        first = False
                        nc.tensor.matmul(pg[:], g_b[:], C[wkey + "b"][:], start=False, stop=False)
                    nc.tensor.matmul(pg[:], ones1[:, :128], C["wg1_h1c"][:], start=False, stop=True)
                    grelu = sb.tile([128, HID], f32, tag="grelu")
                    nc.vector.tensor_scalar_max(grelu[:], pg[:], 0.0)
                    scr2 = sb.tile([128, HID], f32, tag="scrd")
                    gatec = sb.tile([128, 1], f32, tag="gatec")
                    nc.vector.tensor_tensor(out=scr2[:], in0=grelu[:],
                                            in1=C["wg2rep"][:],
                                            op=mybir.AluOpType.mult)
                    nc.vector.tensor_reduce(
                        out=gatec[:], in_=scr2[:].rearrange("p (a b) -> p a b", a=1),
                        axis=mybir.AxisListType.XY, op=mybir.AluOpType.add)
                    ge = sb.tile([128, 1], f32, tag="ge")
                    nc.scalar.activation(out=ge[:], in_=gatec[:],
                                         func=mybir.ActivationFunctionType.Exp,
                                         bias=C["bg2rep"][:, 0:1])
                    sg = sb.tile([128, 32], bf16, tag="sg")
                    nc.vector.tensor_tensor(
                        out=sg[:], in0=batchct[:, w : w + 1].to_broadcast([128, 32]),
                        in1=C["iota32"][:], op=mybir.AluOpType.is_equal)
                    wg = sb.tile([128, JK + 1], bf16, tag="wg")
                    nc.vector.tensor_scalar_mul(wg[:, 0:HID], h1w[:], ge[:, 0:1])
                    nc.vector.tensor_scalar_mul(wg[:, HID:JK], h2w[:], ge[:, 0:1])
                    nc.scalar.copy(out=wg[:, JK : JK + 1], in_=ge[:])
                    return dict(sg=sg, wg=wg)

                def fin_back(w, st):
                    nc.tensor.matmul(pgr[:], st["sg"][:], st["wg"][:], start=(w == 0),
                                     stop=(w == WIN - 1), skip_group_check=True)

                if phases == "full":
                    stp = fin_front(0)
                    for w in range(1, WIN):
                        stn = fin_front(w)
                        fin_back(w - 1, stp)
                        stp = stn
                    fin_back(WIN - 1, stp)

                    pg_sb = sb.tile([32, JK + 1], f32, tag="pg_sb")
                    nc.scalar.copy(out=pg_sb[:], in_=pgr[:])
                    nc.sync.dma_start(out=pool_in[:], in_=pg_sb[:])
                    nc.gpsimd.collective_compute(
                        "AllReduce", mybir.AluOpType.add, replica_groups=rg,
                        ins=[pool_in[:]], outs=[pool_out[:]])
                    psb = sb.tile([32, JK + 1], f32, tag="psb")
                    nc.sync.dma_start(out=psb[:], in_=pool_out[:])
                    gden = sb.tile([32, 1], f32, tag="gden")
                    nc.vector.tensor_scalar_max(gden[:], psb[:, JK : JK + 1], 1e-30)
                    gdr = sb.tile([32, 1], f32, tag="gdr")
                    nc.vector.reciprocal(out=gdr[:], in_=gden[:])
                    pl = sb.tile([32, JK], bf16, tag="pl")
                    nc.vector.tensor_scalar_mul(pl[:], psb[:, 0:JK], gdr[:, 0:1])

                    def _headmm(vin, wa, wb, wc, wd, nout, tagp):
                        pouts = ps.tile([32, nout], f32, tag=tagp,
                                        bufs=(3 if tagp == "qg" else 2))
                        for si, (c0, m) in enumerate(((0, 128), (128, 128), (256, 64))):
                            ptt = ps.tile([m, 32], bf16, tag="tp")
                            nc.tensor.transpose(ptt[:], vin[:, c0 : c0 + m],
                                                C["ident"][0:32, 0:32])
                            tsb = sb.tile([m, 32], bf16, tag="tsb")
                            nc.scalar.copy(out=tsb[:], in_=ptt[:])
                            nc.tensor.matmul(pouts[:], tsb[:], (wa, wb, wc)[si][:m, :],
                                             start=(si == 0), stop=False, skip_group_check=True)
                        nc.tensor.matmul(pouts[:], ones1[:, :32], wd[:],
                                         start=False, stop=True, skip_group_check=True)
                        return pouts

                    ph1 = _headmm(pl, C["wh1a"], C["wh1b"], C["wh1c"], C["wh1d"], JK, "qrb")
                    vrel = sb.tile([32, JK], bf16, tag="vrel")
                    nc.vector.tensor_scalar_max(vrel[:], ph1[:], 0.0)
                    ph2 = _headmm(vrel, C["wh2a"], C["wh2b"], C["wh2c"], C["wh2d"], 6, "qg")
                    osb = sb.tile([32, 6], f32, tag="osb")
                    nc.scalar.copy(out=osb[:], in_=ph2[:])
                    nc.sync.dma_start(out=out_d[:], in_=osb[:])

    nc.compile()
    return nc


_CACHE = {}
_LAST_RES = None


def kernel(**inputs):
    inputs = {k: np.asarray(v) for k, v in inputs.items()}
    per_core, NCH = _preprocess(
        inputs["x"], inputs["edge_index"], inputs["edge_attr"], inputs["batch"])
    w = _weights(inputs)
    import os as _os
    phases = _os.environ.get("KERNEL_PHASES", "full")
    key = (NCH, phases)
    if key not in _CACHE:
        _CACHE[key] = _build(NCH, phases)
    nc = _CACHE[key]
    in_maps = []
    for r in range(NCORES):
        m = dict(w)
        m.update(per_core[r])
        in_maps.append(m)
    import os
    trace = bool(os.environ.get("KERNEL_TRACE"))
    if trace:
        try:
            import axon_prof
            axon_prof.install()
        except Exception:
            trace = False
    res = run_bass_kernel_spmd(nc, in_maps, core_ids=list(range(NCORES)), trace=trace)
    if trace and res.exec_time_ns is not None:
        print(f"HW exec time: {res.exec_time_ns} ns")
        if res.per_core_scope_times:
            for scope, cores in sorted(res.per_core_scope_times.items()):
                print(f"  scope {scope}: {cores}")
    global _LAST_RES
    _LAST_RES = res
    out = res.results[0]["out"]
    return out.reshape(G, 2, 3).astype(np.float32)


# revision 16
# speedup vs baseline: 4.1466x; 1.2051x over previous
"""Trainium2 Bass kernel for nn_EndpointRegressor (2x TransformerConv GNN +
AttentionalAggregation) distributed over 8 NeuronCores.

Sharding: edges partitioned by destination node range (6272 nodes/core);
each core owns its dst nodes exclusively, so segment softmax/scatter stats
need no cross-core reduction.

Layer 0 is gather-free: k0/v0 are low-rank in host-known inputs
(k0[src] = x[src]@(W_in Wk) + ea@We + bias), so alpha0 = sum_c attr10[e,c] *
qW0[dst,h,c] with attr10 = [x[src](5), ea(4), 1] riding the edge stream, and
the value scatter accumulates T[dst,h,c] = sum_e ex*attr10 which is expanded
to 160 dims by one small matmul per 128-node window.  No kv GEMM, no
AllGather, no gather for layer 0.

Layer 1 computes kv for own nodes, AllGathers the [50176,320] bf16 table,
and gathers per-edge rows with indirect_dma_start (int32 indices, HW DGE).
The edge-feature term is folded the same way (qWe trick) so no per-chunk
e-matmul.  Per 128-edge chunk only two PE matmuls remain (q-gather via
one-hot, scatter via one-hot); all per-edge vector math is batched per
128-dst-node window.  Segment softmax uses exp without max subtraction
(alpha ~ +-0.1 for this model family); division by the denominator happens
on the node side.  All matmul operands bf16, PSUM accumulation fp32.

The window loops are software-pipelined (front half of window w+1 emitted
before the back half of window w) so the PE/DVE/Act ping-pong latency of one
window overlaps the next window's independent work.  The node->edge one-hot
(sttw) is host-precomputed and DMAed; the edge->node one-hot (stw) is built
on DVE from bf16 operands.  The only scalar-engine activation functions are
Exp and Copy (one table set, no ACT_TABLE_LOAD thrash); sigmoid is computed
as 1/(1+exp(-z)) with the gate bias pre-negated on the host.
"""
import math
import numpy as np
import ml_dtypes

import concourse.bass as bass
import concourse.bacc as bacc
import concourse.mybir as mybir
import concourse.tile as tile
from concourse._compat import get_trn_type
from concourse.bass_utils import run_bass_kernel_spmd
from concourse.library_config import mlp

# ---- problem constants (fixed by the problem spec) ----
N, E, G = 50000, 500000, 32
H, D = 4, 40
HID = H * D            # 160
JK = 2 * HID           # 320
NCORES = 8
NSHARD = 6272          # 49*128 nodes per core
NPAD = NCORES * NSHARD # 50176
WIN = NSHARD // 128    # 49
SEGW = 7               # windows per AllGather segment (49 = 7*7)
SEGR = SEGW * 128      # 896 rows per core per segment
INVSQD = 1.0 / math.sqrt(float(D))

f32 = mybir.dt.float32
bf16 = mybir.dt.bfloat16
i32 = mybir.dt.int32
nbf = ml_dtypes.bfloat16


def _preprocess(x, edge_index, edge_attr, batch):
    """Sort edges by dst, shard by dst range, pad each (core,window) bucket
    to NCH chunks of 128 edge slots. Build the per-edge attribute streams."""
    src = np.asarray(edge_index[0], dtype=np.int64)
    dst = np.asarray(edge_index[1], dtype=np.int64)
    ea = np.asarray(edge_attr, dtype=np.float32)
    x = np.asarray(x, np.float32)
    order = np.argsort(dst, kind="stable")
    src, dst, ea = src[order], dst[order], ea[order]

    core = dst // NSHARD
    win = (dst % NSHARD) // 128

    buckets = {}
    for r in range(NCORES):
        m_r = core == r
        for w in range(WIN):
            buckets[(r, w)] = np.nonzero(m_r & (win == w))[0]

    NCH = max(1, max((len(b) + 127) // 128 for b in buckets.values()))

    per_core = []
    for r in range(NCORES):
        idxE = np.zeros((128, WIN * NCH), np.int32)
        attr0 = np.zeros((128, WIN * NCH * 40), np.float32)
        eaQ1 = np.zeros((128, WIN * NCH * 20), np.float32)
        drwC = np.full((128, WIN * NCH), -1.0, np.float32)
        drwR = np.full((WIN, NCH * 128), -1.0, np.float32)
        for w in range(WIN):
            eidx = buckets[(r, w)]
            n = len(eidx)
            kk = np.arange(n)
            jj = kk // 128
            pp = kk % 128
            col = w * NCH + jj
            # kv_full is segment-major: [seg, core, 896, 320]
            sg_ = (src[eidx] % NSHARD) // SEGR
            idxE[pp, col] = (sg_ * NCORES * SEGR
                             + (src[eidx] // NSHARD) * SEGR
                             + (src[eidx] % NSHARD) % SEGR).astype(np.int64)
            dd = (dst[eidx] % 128).astype(np.float32)
            drwC[pp, col] = dd
            drwR[w, jj * 128 + pp] = dd
            # attr10 = [x[src](5), ea(4), 1] replicated per head
            a10 = np.concatenate(
                [x[src[eidx]], ea[eidx], np.ones((n, 1), np.float32)], 1)  # [n,10]
            ea5 = np.concatenate([ea[eidx], np.ones((n, 1), np.float32)], 1)  # [n,5]
            for h in range(H):
                c0 = col * 40 + h * 10
                for c in range(10):
                    attr0[pp, c0 + c] = a10[:, c]
                c1 = col * 20 + h * 5
                for c in range(5):
                    eaQ1[pp, c1 + c] = ea5[:, c]
        n0 = r * NSHARD
        xT6 = np.zeros((6, NSHARD), np.float32)
        xT6[5, :] = 1.0
        batchc = np.full((128, WIN), -1.0, np.float32)
        n_real = max(0, min(NSHARD, N - n0))
        if n_real > 0:
            xT6[:5, :n_real] = x[n0 : n0 + n_real].T
            bfull = np.full(NSHARD, -1.0, np.float32)
            bfull[:n_real] = np.asarray(batch[n0 : n0 + n_real], np.float32)
            batchc[:, :] = bfull.reshape(WIN, 128).T
        # host-built one-hot: sttw[w, p, e] = (dst%128 of edge e == p)
        sttw = (drwR[:, None, :] == np.arange(128, dtype=np.float32)[None, :, None])
        per_core.append(
            dict(
                xT6=xT6.astype(nbf),
                idxE=idxE,
                attr0=attr0.astype(nbf),
                eaQ1=eaQ1.astype(nbf),
                drwC=drwC.astype(nbf),
                sttw=sttw.astype(nbf),
                batchc=batchc.astype(nbf),
            )
        )
    return per_core, NCH


def _weights(inp):
    """Host-side weight packing/folding (fp64 math, bf16 output)."""
    w = {}
    f8 = np.float64
    W_in = inp["W_in"].astype(f8)
    b_in = inp["b_in"].astype(f8)

    w["iota128"] = np.broadcast_to(np.arange(128, dtype=np.float32), (128, 128)).astype(nbf).copy()
    w["iota32"] = np.broadcast_to(np.arange(32, dtype=np.float32), (128, 32)).astype(nbf).copy()
    w["ident"] = np.eye(128, dtype=np.float32).astype(nbf)
    w["identf"] = np.eye(128, dtype=np.float32)
    Wg1 = inp["Wg1"].astype(np.float32)
    w["wg1_h1"] = np.concatenate([Wg1[:HID], inp["bg1"].astype(np.float32)[None, :]], 0).astype(nbf)
    w["wg1_h2"] = np.concatenate([Wg1[HID:], np.zeros((1, HID), np.float32)], 0).astype(nbf)
    w["wg2rep"] = np.broadcast_to(inp["Wg2"].astype(np.float32)[:, 0], (128, HID)).copy()
    w["wh1"] = np.concatenate([inp["Wh1"].astype(np.float32), inp["bh1"].astype(np.float32)[None, :]], 0).astype(nbf)
    w["wh2"] = np.concatenate([inp["Wh2"].astype(np.float32), inp["bh2"].astype(np.float32)[None, :]], 0).astype(nbf)
    w["bg2rep"] = np.full((128, 1), float(np.asarray(inp["bg2"]).reshape(-1)[0]), np.float32)

    for layer in range(2):
        Wq, Wk, Wv = (inp[k][layer].astype(f8) for k in ("Wq", "Wk", "Wv"))
        bq, bk, bv = (inp[k][layer].astype(f8) for k in ("bq", "bk", "bv"))
        Wskip, bskip = inp["Wskip"][layer].astype(f8), inp["bskip"][layer].astype(f8)
        Wbeta = inp["Wbeta"][layer].astype(f8)
        We, be = inp["We"][layer].astype(f8), inp["be"][layer].astype(f8)
        P = (Wbeta[:HID, 0] + Wbeta[2 * HID :, 0])
        Q = (Wbeta[HID : 2 * HID, 0] - Wbeta[2 * HID :, 0])
        w[f"prep{layer}"] = np.broadcast_to(P.astype(np.float32), (128, HID)).copy()
        if layer == 0:
            Q6 = np.concatenate([W_in @ Wq, (b_in @ Wq + bq)[None, :]], 0)     # [6,160]
            K10 = np.concatenate([W_in @ Wk, We, (b_in @ Wk + bk + be)[None, :]], 0)  # [10,160]
            V10 = np.concatenate([W_in @ Wv, We, (b_in @ Wv + bv + be)[None, :]], 0)  # [10,160]
            wq0x = np.zeros((6, 40), f8)
            wvblk0 = np.zeros((40, HID), f8)
            for h in range(H):
                ds = slice(h * D, (h + 1) * D)
                wq0x[:, h * 10 : (h + 1) * 10] = np.einsum(
                    "fd,cd->fc", Q6[:, ds], K10[:, ds])
                wvblk0[h * 10 : (h + 1) * 10, ds] = V10[:, ds]
            S6 = np.concatenate([W_in @ Wskip, (b_in @ Wskip + bskip)[None, :]], 0)  # [6,160]
            wrb0 = np.concatenate([S6, -(S6 @ Q)[:, None]], 1)            # [6,161]
            w["wq0x"] = wq0x.astype(np.float32).astype(nbf)
            w["wvblk0"] = wvblk0.astype(np.float32).astype(nbf)
            w["wrb0"] = wrb0.astype(np.float32).astype(nbf)
        else:
            We5 = np.concatenate([We, be[None, :]], 0)                    # [5,160]
            Q161 = np.concatenate([Wq, bq[None, :]], 0)                   # [161,160]
            wq1x = np.zeros((161, 180), f8)
            weblk1 = np.zeros((20, HID), f8)
            for h in range(H):
                ds = slice(h * D, (h + 1) * D)
                wq1x[:, h * 45 : h * 45 + 40] = Q161[:, ds]
                wq1x[:, h * 45 + 40 : h * 45 + 45] = np.einsum(
                    "fd,cd->fc", Q161[:, ds], We5[:, ds])
                weblk1[h * 5 : (h + 1) * 5, ds] = We5[:, ds]
            S161 = np.concatenate([Wskip, bskip[None, :]], 0)             # [161,160]
            wrb1 = np.concatenate([S161, -(S161 @ Q)[:, None]], 1)        # [161,161]
            wkv1 = np.concatenate(
                [np.concatenate([Wk, Wv], 1), np.concatenate([bk, bv])[None, :]], 0)  # [161,320]
            w["wq1x"] = wq1x.astype(np.float32).astype(nbf)
            w["weblk1"] = weblk1.astype(np.float32).astype(nbf)
            w["wrb1"] = wrb1.astype(np.float32).astype(nbf)
            w["wkv1"] = wkv1.astype(np.float32).astype(nbf)
    return w


def _build(NCH, phases="full"):
    nc = bacc.Bacc(get_trn_type() or "TRN2", target_bir_lowering=False)
    NE = NCH * 128

    # ---- dram I/O ----
    d = {}
    d["xT6"] = nc.dram_tensor("xT6", [6, NSHARD], bf16, kind="ExternalInput")
    d["idxE"] = nc.dram_tensor("idxE", [128, WIN * NCH], i32, kind="ExternalInput")
    d["attr0"] = nc.dram_tensor("attr0", [128, WIN * NCH * 40], bf16, kind="ExternalInput")
    d["eaQ1"] = nc.dram_tensor("eaQ1", [128, WIN * NCH * 20], bf16, kind="ExternalInput")
    d["drwC"] = nc.dram_tensor("drwC", [128, WIN * NCH], bf16, kind="ExternalInput")
    d["sttw"] = nc.dram_tensor("sttw", [WIN, 128, NE], bf16, kind="ExternalInput")
    d["batchc"] = nc.dram_tensor("batchc", [128, WIN], bf16, kind="ExternalInput")
    wshapes = dict(
        iota128=([128, 128], bf16), iota32=([128, 32], bf16),
        ident=([128, 128], bf16), identf=([128, 128], f32),
        wg1_h1=([161, HID], bf16), wg1_h2=([161, HID], bf16), wg2rep=([128, HID], f32),
        wh1=([321, JK], bf16), wh2=([321, 6], bf16), bg2rep=([128, 1], f32),
        prep0=([128, HID], f32), prep1=([128, HID], f32),
        wq0x=([6, 40], bf16), wvblk0=([40, HID], bf16), wrb0=([6, 161], bf16),
        wq1x=([161, 180], bf16), weblk1=([20, HID], bf16), wrb1=([161, 161], bf16),
        wkv1=([161, JK], bf16),
    )
    for k, (shp, dt_) in wshapes.items():
        d[k] = nc.dram_tensor(k, shp, dt_, kind="ExternalInput")
    out_d = nc.dram_tensor("out", [32, 6], f32, kind="ExternalOutput")

    hT = [nc.dram_tensor(f"hT{i}", [HID, NSHARD], bf16) for i in (1, 2)]
    h_nm = [nc.dram_tensor(f"h_nm{i}", [NSHARD, HID], bf16) for i in (1, 2)]
    kv_own = nc.dram_tensor("kv_own1", [NSHARD, JK], bf16)
    kv_full = nc.dram_tensor("kv_full1", [NPAD, JK], bf16, addr_space="Shared")
    pool_in = nc.dram_tensor("pool_in", [32, JK + 1], f32)
    pool_out = nc.dram_tensor("pool_out", [32, JK + 1], f32, addr_space="Shared")
    rg = [list(range(NCORES))]

    with tile.TileContext(nc) as tc:
        with (
            tc.tile_pool(name="cst", bufs=1) as cst,
            tc.tile_pool(name="sb", bufs=2) as sb,
            tc.tile_pool(name="gath", bufs=3) as gath,
            tc.tile_pool(name="ps", bufs=2, space="PSUM") as ps,
        ):
            nc.gpsimd.load_library(mlp)

            # ---- persistent constants + preloads ----
            C = {}
            def _load_const(key, part, cols, row0=0):
                t = cst.tile([part, cols], wshapes[key][1], name=f"c_{key}_{row0}")
                nc.sync.dma_start(out=t[:], in_=d[key][row0 : row0 + part, :])
                return t
            for key in ("iota128", "iota32", "ident", "identf", "wg2rep",
                        "bg2rep", "prep0", "prep1", "wvblk0", "weblk1"):
                C[key] = _load_const(key, wshapes[key][0][0], wshapes[key][0][1])
            C["wq0x"] = _load_const("wq0x", 6, 40)
            C["wrb0"] = _load_const("wrb0", 6, 161)
            for key, cols in (("wq1x", 180), ("wrb1", 161), ("wkv1", JK),
                              ("wg1_h1", HID), ("wg1_h2", HID)):
                C[key + "a"] = _load_const(key, 128, cols)
                C[key + "b"] = _load_const(key, 32, cols, 128)
                C[key + "c"] = _load_const(key, 1, cols, 160)
            C["wh1a"] = _load_const("wh1", 128, JK)
            C["wh1b"] = _load_const("wh1", 128, JK, 128)
            C["wh1c"] = _load_const("wh1", 64, JK, 256)
            C["wh1d"] = _load_const("wh1", 1, JK, 320)
            C["wh2a"] = _load_const("wh2", 128, 6)
            C["wh2b"] = _load_const("wh2", 128, 6, 128)
            C["wh2c"] = _load_const("wh2", 64, 6, 256)
            C["wh2d"] = _load_const("wh2", 1, 6, 320)

            idxt = cst.tile([128, WIN * NCH], i32, name="idxt")
            nc.sync.dma_start(out=idxt[:], in_=d["idxE"][:])
            attr0t = cst.tile([128, WIN * NCH * 40], bf16, name="attr0t")
            nc.sync.dma_start(out=attr0t[:], in_=d["attr0"][:])
            eaQ1t = cst.tile([128, WIN * NCH * 20], bf16, name="eaQ1t")
            nc.sync.dma_start(out=eaQ1t[:], in_=d["eaQ1"][:])
            drwCt = cst.tile([128, WIN * NCH], bf16, name="drwCt")
            nc.sync.dma_start(out=drwCt[:], in_=d["drwC"][:])
            batchct = cst.tile([128, WIN], bf16, name="batchct")
            nc.sync.dma_start(out=batchct[:], in_=d["batchc"][:])
            xt6t = cst.tile([6, NSHARD], bf16, name="xt6t")
            nc.sync.dma_start(out=xt6t[:], in_=d["xT6"][:])

            ones1 = cst.tile([1, 128], bf16, name="ones1")
            nc.gpsimd.memset(ones1[:], 1.0)
            # per-window scatter accumulator bank + separate persistent
            # cross-window pool accumulator bank
            acct_t = ps.tile([128, 204], f32, tag="acc", bufs=2)
            acct = acct_t[:]
            pgr_t = ps.tile([32, JK + 1], f32, tag="acc", bufs=2)
            pgr = pgr_t[:]

            def _masks(w):
                """stw (edge p -> node one-hot) on DVE; sttw host-DMAed."""
                stw = sb.tile([128, NE], bf16, tag="stw")
                nc.vector.tensor_tensor(
                    out=stw[:].rearrange("p (c x) -> p c x", x=128),
                    in0=drwCt[:, w * NCH : (w + 1) * NCH]
                        .rearrange("p (c o) -> p c o", o=1)
                        .to_broadcast([128, NCH, 128]),
                    in1=C["iota128"][:].rearrange("p (o x) -> p o x", o=1)
                        .to_broadcast([128, NCH, 128]),
                    op=mybir.AluOpType.is_equal)
                sttw = sb.tile([128, NE], bf16, tag="sttw")
                nc.sync.dma_start(out=sttw[:], in_=d["sttw"][w])
                return stw, sttw

            def _post_tail(layer, w, vsrc, rsb, denr, store=True):
                """outn -> beta gate -> h' -> transposes; returns (hp, t1, t2)."""
                outn = sb.tile([128, HID], f32, tag="outn")
                nc.vector.tensor_tensor(
                    out=outn[:].rearrange("p (h dd) -> p h dd", h=H),
                    in0=vsrc.rearrange("p (h dd) -> p h dd", h=H),
                    in1=denr[:].rearrange("p (h o) -> p h o", o=1).to_broadcast([128, H, D]),
                    op=mybir.AluOpType.mult)
                scrd = sb.tile([128, HID], f32, tag="scrd")
                outP = sb.tile([128, 1], f32, tag="outP")
                nc.vector.tensor_tensor(out=scrd[:], in0=outn[:],
                                        in1=C[f"prep{layer}"][:],
                                        op=mybir.AluOpType.mult)
                nc.vector.tensor_reduce(
                    out=outP[:], in_=scrd[:].rearrange("p (a b) -> p a b", a=1),
                    axis=mybir.AxisListType.XY, op=mybir.AluOpType.add)
                u = sb.tile([128, 1], f32, tag="u")
                nc.scalar.activation(out=u[:], in_=outP[:],
                                     func=mybir.ActivationFunctionType.Exp,
                                     scale=-1.0, bias=rsb[:, 160:161])
                up1 = sb.tile([128, 1], f32, tag="up1")
                nc.vector.tensor_scalar_add(up1[:], u[:], 1.0)
                beta = sb.tile([128, 1], f32, tag="beta")
                nc.vector.reciprocal(out=beta[:], in_=up1[:])
                dvec = sb.tile([128, HID], f32, tag="dvec")
                nc.vector.tensor_sub(dvec[:], rsb[:, :HID], outn[:])
                hp = sb.tile([128, HID], bf16, tag="hp")
                nc.vector.scalar_tensor_tensor(
                    out=hp[:], in0=dvec[:], scalar=beta[:, 0:1], in1=outn[:],
                    op0=mybir.AluOpType.mult, op1=mybir.AluOpType.add)
                wsl = slice(w * 128, (w + 1) * 128)
                if store:
                    nc.sync.dma_start(out=h_nm[layer][wsl, :], in_=hp[:])
                ptr1 = ps.tile([128, 128], bf16, tag="tp")
                nc.tensor.transpose(ptr1[:], hp[:, 0:128], C["ident"][:])
                t1 = sb.tile([128, 128], bf16, tag="t1")
                nc.scalar.copy(out=t1[:], in_=ptr1[:])
                ptr2 = ps.tile([32, 128], bf16, tag="tp")
                nc.tensor.transpose(ptr2[:], hp[:, 128:160], C["ident"][:])
                t2 = sb.tile([32, 128], bf16, tag="t2")
                nc.scalar.copy(out=t2[:], in_=ptr2[:])
                if store:
                    nc.sync.dma_start(out=hT[layer][0:128, wsl], in_=t1[:])
                    nc.sync.dma_start(out=hT[layer][128:160, wsl], in_=t2[:])
                return hp, t1, t2

            # ==== layer 0 (gather-free) + kv1 GEMM fold-in, pipelined ====
            def l0_front(w):
                wsl = slice(w * 128, (w + 1) * 128)
                stw, sttw = _masks(w)
                pqrb = ps.tile([128, 201], f32, tag="qrb", bufs=2)
                nc.tensor.matmul(pqrb[:, 0:40], xt6t[:, wsl], C["wq0x"][:],
                                 start=True, stop=True, skip_group_check=True)
                nc.tensor.matmul(pqrb[:, 40:201], xt6t[:, wsl], C["wrb0"][:],
                                 start=True, stop=True, skip_group_check=True)
                qx = sb.tile([128, 40], bf16, tag="qx")
                nc.scalar.copy(out=qx[:], in_=pqrb[:, 0:40])
                rsb = sb.tile([128, 161], f32, tag="rsb")
                nc.scalar.copy(out=rsb[:], in_=pqrb[:, 40:201])
                pqg = ps.tile([128, NCH * 40], f32, tag="qg", bufs=2)
                for j in range(NCH):
                    nc.tensor.matmul(pqg[:, j * 40 : (j + 1) * 40],
                                     sttw[:, j * 128 : (j + 1) * 128], qx[:],
                                     start=True, stop=True, skip_group_check=True)
                a0sl = slice(w * NCH * 40, (w + 1) * NCH * 40)
                tqw0 = sb.tile([128, NCH * 40], bf16, tag="tqw0")
                nc.vector.tensor_tensor(out=tqw0[:], in0=pqg[:],
                                        in1=attr0t[:, a0sl],
                                        op=mybir.AluOpType.mult)
                alf = sb.tile([128, NCH * H], f32, tag="alf")
                nc.vector.tensor_reduce(
                    out=alf[:], in_=tqw0[:].rearrange("p (c x) -> p c x", x=10),
                    axis=mybir.AxisListType.X, op=mybir.AluOpType.add)
                exw = sb.tile([128, NCH * H], bf16, tag="exw")
                nc.scalar.activation(out=exw[:], in_=alf[:],
                                     func=mybir.ActivationFunctionType.Exp,
                                     scale=INVSQD)
                wt0 = sb.tile([128, NCH, 44], bf16, tag="wt0")
                nc.vector.tensor_tensor(
                    out=wt0[:, :, 0:40].rearrange("p c (h x) -> p c h x", h=H),
                    in0=attr0t[:, a0sl].rearrange("p (c h x) -> p c h x", h=H, x=10),
                    in1=exw[:].rearrange("p (c h o) -> p c h o", h=H, o=1)
                        .to_broadcast([128, NCH, H, 10]),
                    op=mybir.AluOpType.mult)
                nc.scalar.copy(out=wt0[:, :, 40:44],
                               in_=exw[:].rearrange("p (c x) -> p c x", x=H))
                return dict(stw=stw, wt0=wt0, rsb=rsb)

            def l0_back(w, st):
                wsl = slice(w * 128, (w + 1) * 128)
                pacc = acct
                for j in range(NCH):
                    nc.tensor.matmul(pacc[:, 0:44],
                                     st["stw"][:, j * 128 : (j + 1) * 128],
                                     st["wt0"][:, j, :], start=(j == 0),
                                     stop=(j == NCH - 1), skip_group_check=True)
                accsb = sb.tile([128, 44], f32, tag="accsb")
                nc.scalar.copy(out=accsb[:], in_=pacc[:, 0:44])
                dmax = sb.tile([128, H], f32, tag="dmax")
                nc.vector.tensor_scalar_max(dmax[:], accsb[:, 40:44], 1e-30)
                denr = sb.tile([128, H], f32, tag="denr")
                nc.vector.reciprocal(out=denr[:], in_=dmax[:])
                ptt = ps.tile([40, 128], f32, tag="tp")
                nc.tensor.transpose(ptt[:], accsb[:, 0:40], C["identf"][:])
                t40 = sb.tile([40, 128], bf16, tag="t40")
                nc.scalar.copy(out=t40[:], in_=ptt[:])
                nc.tensor.matmul(pacc[:, 44:204], t40[:], C["wvblk0"][:],
                                 start=True, stop=True, skip_group_check=True)
                hp, t1, t2 = _post_tail(0, w, pacc[:, 44:204], st["rsb"], denr)
                pkv = ps.tile([128, JK], f32, tag="qg", bufs=2)
                nc.tensor.matmul(pkv[:], t1[:], C["wkv1a"][:], start=True, stop=False)
                nc.tensor.matmul(pkv[:], t2[:], C["wkv1b"][:], start=False, stop=False)
                nc.tensor.matmul(pkv[:], ones1[:, :128], C["wkv1c"][:], start=False, stop=True)
                kvsb = sb.tile([128, JK], bf16, tag="kvsb")
                nc.scalar.copy(out=kvsb[:], in_=pkv[:])
                nc.sync.dma_start(out=kv_own[wsl, :], in_=kvsb[:])

            do_ag = phases in ("ag", "l1nog", "l1", "full")

            def _ag_seg(sg_):
                s0 = sg_ * SEGR
                s1 = min((sg_ + 1) * SEGR, NSHARD)
                o0 = sg_ * NCORES * SEGR
                nc.gpsimd.collective_compute(
                    "AllGather", mybir.AluOpType.bypass, replica_groups=rg,
                    ins=[kv_own[s0:s1, :]],
                    outs=[kv_full[o0 : o0 + NCORES * (s1 - s0), :]])

            NW0 = 2 if phases == "mini" else WIN
            nseg = 0
            with nc.named_scope("l0"):
                stp = l0_front(0)
                for w in range(1, NW0):
                    stn = l0_front(w)
                    l0_back(w - 1, stp)
                    stp = stn
                    while do_ag and (nseg + 1) * SEGW <= w:
                        _ag_seg(nseg)
                        nseg += 1
                l0_back(NW0 - 1, stp)
            if do_ag:
                with nc.named_scope("ag1"):
                    while nseg * SEGR < NSHARD:
                        _ag_seg(nseg)
                        nseg += 1

            # ==== layer 1, pipelined ====
            def l1_front(w):
                wsl = slice(w * 128, (w + 1) * 128)
                kvw = gath.tile([128, NCH, JK], bf16, tag="kvw")
                if phases == "l1nog":
                    nc.gpsimd.memset(kvw[:], 0.0)
                else:
                    for j in range(NCH):
                        nc.gpsimd.indirect_dma_start(
                            out=kvw[:, j, :], out_offset=None,
                            in_=kv_full[:],
                            in_offset=bass.IndirectOffsetOnAxis(
                                ap=idxt[:, w * NCH + j : w * NCH + j + 1], axis=0),
                        )
                hta = sb.tile([128, 128], bf16, tag="hta", bufs=3)
                nc.sync.dma_start(out=hta[:], in_=hT[0][0:128, wsl])
                htb = sb.tile([32, 128], bf16, tag="htb", bufs=3)
                nc.sync.dma_start(out=htb[:], in_=hT[0][128:160, wsl])
                stw, sttw = _masks(w)
                pqrb = ps.tile([128, 341], f32, tag="qrb", bufs=2)
                nc.tensor.matmul(pqrb[:, 0:180], hta[:], C["wq1xa"][:],
                                 start=True, stop=False, skip_group_check=True)
                nc.tensor.matmul(pqrb[:, 0:180], htb[:], C["wq1xb"][:],
                                 start=False, stop=False, skip_group_check=True)
                nc.tensor.matmul(pqrb[:, 0:180], ones1[:, :128], C["wq1xc"][:],
                                 start=False, stop=True, skip_group_check=True)
                nc.tensor.matmul(pqrb[:, 180:341], hta[:], C["wrb1a"][:],
                                 start=True, stop=False, skip_group_check=True)
                nc.tensor.matmul(pqrb[:, 180:341], htb[:], C["wrb1b"][:],
                                 start=False, stop=False, skip_group_check=True)
                nc.tensor.matmul(pqrb[:, 180:341], ones1[:, :128], C["wrb1c"][:],
                                 start=False, stop=True, skip_group_check=True)
                qx = sb.tile([128, 180], bf16, tag="qx")
                nc.scalar.copy(out=qx[:], in_=pqrb[:, 0:180])
                rsb = sb.tile([128, 161], f32, tag="rsb")
                nc.scalar.copy(out=rsb[:], in_=pqrb[:, 180:341])
                tqw = sb.tile([128, NCH * 180], bf16, tag="tqw")
                e1sl = w * NCH * 20
                for g in range((NCH + 1) // 2):
                    j0 = 2 * g
                    jn = min(2, NCH - j0)
                    pqg = ps.tile([128, 360], f32, tag="qg", bufs=2)
                    for jj in range(jn):
                        nc.tensor.matmul(
                            pqg[:, jj * 180 : (jj + 1) * 180],
                            sttw[:, (j0 + jj) * 128 : (j0 + jj + 1) * 128],
                            qx[:], start=True, stop=True, skip_group_check=True)
                    tq4 = tqw[:].rearrange("p (c h x) -> p c h x", h=H, x=45)
                    pq4 = pqg[:, 0 : jn * 180].rearrange(
                        "p (c h x) -> p c h x", h=H, x=45)
                    nc.vector.tensor_tensor(
                        out=tq4[:, j0 : j0 + jn, :, 0:40],
                        in0=pq4[:, :, :, 0:40],
                        in1=kvw[:, j0 : j0 + jn, 0:HID]
                            .rearrange("p c (h dd) -> p c h dd", h=H),
                        op=mybir.AluOpType.mult)
                    nc.vector.tensor_tensor(
                        out=tq4[:, j0 : j0 + jn, :, 40:45],
                        in0=pq4[:, :, :, 40:45],
                        in1=eaQ1t[:, e1sl + j0 * 20 : e1sl + (j0 + jn) * 20]
                            .rearrange("p (c h x) -> p c h x", h=H, x=5),
                        op=mybir.AluOpType.mult)
                alf = sb.tile([128, NCH * H], f32, tag="alf")
                nc.vector.tensor_reduce(
                    out=alf[:], in_=tqw[:].rearrange("p (c x) -> p c x", x=45),
                    axis=mybir.AxisListType.X, op=mybir.AluOpType.add)
                exw = sb.tile([128, NCH * H], bf16, tag="exw")
                nc.scalar.activation(out=exw[:], in_=alf[:],
                                     func=mybir.ActivationFunctionType.Exp,
                                     scale=INVSQD)
                wt = sb.tile([128, NCH, 184], bf16, tag="wt")
                nc.vector.tensor_tensor(
                    out=wt[:, :, 0:HID].rearrange("p c (h dd) -> p c h dd", h=H),
                    in0=kvw[:, :, HID:JK].rearrange("p c (h dd) -> p c h dd", h=H),
                    in1=exw[:].rearrange("p (c h o) -> p c h o", h=H, o=1)
                        .to_broadcast([128, NCH, H, D]),
                    op=mybir.AluOpType.mult)
                nc.scalar.copy(out=wt[:, :, 160:164],
                               in_=exw[:].rearrange("p (c x) -> p c x", x=H))
                nc.vector.tensor_tensor(
                    out=wt[:, :, 164:184].rearrange("p c (h x) -> p c h x", h=H),
                    in0=eaQ1t[:, e1sl : e1sl + NCH * 20]
                        .rearrange("p (c h x) -> p c h x", h=H, x=5),
                    in1=exw[:].rearrange("p (c h o) -> p c h o", h=H, o=1)
                        .to_broadcast([128, NCH, H, 5]),
                    op=mybir.AluOpType.mult)
                return dict(stw=stw, wt=wt, rsb=rsb, hta=hta, htb=htb)

            def l1_back(w, st):
                wsl = slice(w * 128, (w + 1) * 128)
                pacc = acct
                for j in range(NCH):
                    nc.tensor.matmul(pacc[:, 0:184],
                                     st["stw"][:, j * 128 : (j + 1) * 128],
                                     st["wt"][:, j, :], start=(j == 0),
                                     stop=(j == NCH - 1), skip_group_check=True)
                accsb = sb.tile([128, 184], f32, tag="accsb")
                nc.scalar.copy(out=accsb[:], in_=pacc[:, 0:184])
                dmax = sb.tile([128, H], f32, tag="dmax")
                nc.vector.tensor_scalar_max(dmax[:], accsb[:, 160:164], 1e-30)
                denr = sb.tile([128, H], f32, tag="denr")
                nc.vector.reciprocal(out=denr[:], in_=dmax[:])
                ptt = ps.tile([20, 128], f32, tag="tp")
                nc.tensor.transpose(ptt[:], accsb[:, 164:184], C["identf"][:])
                t20 = sb.tile([20, 128], bf16, tag="t40")
                nc.scalar.copy(out=t20[:], in_=ptt[:])
                # e-contribution accumulates straight onto the v-sums in PSUM
                nc.tensor.matmul(pacc[:, 0:HID], t20[:], C["weblk1"][:],
                                 start=False, stop=True, skip_group_check=True)
                hp, t1, t2 = _post_tail(1, w, pacc[:, 0:HID], st["rsb"], denr,
                                        store=False)
                # ---- fused AttentionalAggregation gate + pooled scatter ----
                h1w = sb.tile([128, HID], bf16, tag="h1w")
                nc.sync.dma_start(out=h1w[:], in_=h_nm[0][wsl, :])
                pg = ps.tile([128, HID], f32, tag="qg", bufs=2)
                nc.tensor.matmul(pg[:], st["hta"][:], C["wg1_h1a"][:], start=True, stop=False)
                nc.tensor.matmul(pg[:], st["htb"][:], C["wg1_h1b"][:], start=False, stop=False)
                nc.tensor.matmul(pg[:], t1[:], C["wg1_h2a"][:], start=False, stop=False)
                nc.tensor.matmul(pg[:], t2[:], C["wg1_h2b"][:], start=False, stop=False)
                nc.tensor.matmul(pg[:], ones1[:, :128], C["wg1_h1c"][:], start=False, stop=True)
                grelu = sb.tile([128, HID], f32, tag="grelu")
                nc.vector.tensor_scalar_max(grelu[:], pg[:], 0.0)
                scr2 = sb.tile([128, HID], f32, tag="scr2")
                gatec = sb.tile([128, 1], f32, tag="gatec")
                nc.vector.tensor_tensor(out=scr2[:], in0=grelu[:],
                                        in1=C["wg2rep"][:],
                                        op=mybir.AluOpType.mult)
                nc.vector.tensor_reduce(
                    out=gatec[:], in_=scr2[:].rearrange("p (a b) -> p a b", a=1),
                    axis=mybir.AxisListType.XY, op=mybir.AluOpType.add)
                ge = sb.tile([128, 1], f32, tag="ge")
                nc.scalar.activation(out=ge[:], in_=gatec[:],
                                     func=mybir.ActivationFunctionType.Exp,
                                     bias=C["bg2rep"][:, 0:1])
                sg = sb.tile([128, 32], bf16, tag="sg")
                nc.vector.tensor_tensor(
                    out=sg[:], in0=batchct[:, w : w + 1].to_broadcast([128, 32]),
                    in1=C["iota32"][:], op=mybir.AluOpType.is_equal)
                wg = sb.tile([128, JK + 1], bf16, tag="wg")
                nc.vector.tensor_scalar_mul(wg[:, 0:HID], h1w[:], ge[:, 0:1])
                nc.vector.tensor_scalar_mul(wg[:, HID:JK], hp[:], ge[:, 0:1])
                nc.scalar.copy(out=wg[:, JK : JK + 1], in_=ge[:])
                nc.tensor.matmul(pgr, sg[:], wg[:], start=(w == 0),
                                 stop=(w == WIN - 1), skip_group_check=True)

            with nc.named_scope("l1"):
                if phases in ("l1nog", "l1", "full"):
                    stp = l1_front(0)
                    for w in range(1, WIN):
                        stn = l1_front(w)
                        l1_back(w - 1, stp)
                        stp = stn
                    l1_back(WIN - 1, stp)

            # ==== final phase: gate + graph pooling + head MLP ====
            if phases != "full":
                dummy = sb.tile([32, 6], f32, tag="osb")
                nc.gpsimd.memset(dummy[:], 0.0)
                nc.sync.dma_start(out=out_d[:], in_=dummy[:])
            with nc.named_scope("final"):
                if phases == "full":
                    pg_sb = sb.tile([32, JK + 1], f32, tag="pg_sb")
                    nc.scalar.copy(out=pg_sb[:], in_=pgr[:])
                    nc.sync.dma_start(out=pool_in[:], in_=pg_sb[:])
                    nc.gpsimd.collective_compute(
                        "AllReduce", mybir.AluOpType.add, replica_groups=rg,
                        ins=[pool_in[:]], outs=[pool_out[:]])
                    psb = sb.tile([32, JK + 1], f32, tag="psb")
                    nc.sync.dma_start(out=psb[:], in_=pool_out[:])
                    gden = sb.tile([32, 1], f32, tag="gden")
                    nc.vector.tensor_scalar_max(gden[:], psb[:, JK : JK + 1], 1e-30)
                    gdr = sb.tile([32, 1], f32, tag="gdr")
                    nc.vector.reciprocal(out=gdr[:], in_=gden[:])
                    pl = sb.tile([32, JK], bf16, tag="pl")
                    nc.vector.tensor_scalar_mul(pl[:], psb[:, 0:JK], gdr[:, 0:1])

                    def _headmm(vin, wa, wb, wc, wd, nout, tagp):
                        pouts = ps.tile([32, nout], f32, tag=tagp, bufs=2)
                        for si, (c0, m) in enumerate(((0, 128), (128, 128), (256, 64))):
                            ptt = ps.tile([m, 32], bf16, tag="tp")
                            nc.tensor.transpose(ptt[:], vin[:, c0 : c0 + m],
                                                C["ident"][0:32, 0:32])
                            tsb = sb.tile([m, 32], bf16, tag="tsb")
                            nc.scalar.copy(out=tsb[:], in_=ptt[:])
                            nc.tensor.matmul(pouts[:], tsb[:], (wa, wb, wc)[si][:m, :],
                                             start=(si == 0), stop=False, skip_group_check=True)
                        nc.tensor.matmul(pouts[:], ones1[:, :32], wd[:],
                                         start=False, stop=True, skip_group_check=True)
                        return pouts

                    ph1 = _headmm(pl, C["wh1a"], C["wh1b"], C["wh1c"], C["wh1d"], JK, "qrb")
                    vrel = sb.tile([32, JK], bf16, tag="vrel")
                    nc.vector.tensor_scalar_max(vrel[:], ph1[:], 0.0)
                    ph2 = _headmm(vrel, C["wh2a"], C["wh2b"], C["wh2c"], C["wh2d"], 6, "qg")
                    osb = sb.tile([32, 6], f32, tag="osb")
                    nc.scalar.copy(out=osb[:], in_=ph2[:])
                    nc.sync.dma_start(out=out_d[:], in_=osb[:])

    nc.compile()
    return nc


_CACHE = {}
_LAST_RES = None


def kernel(**inputs):
    inputs = {k: np.asarray(v) for k, v in inputs.items()}
    per_core, NCH = _preprocess(
        inputs["x"], inputs["edge_index"], inputs["edge_attr"], inputs["batch"])
    w = _weights(inputs)
    import os as _os
    phases = _os.environ.get("KERNEL_PHASES", "full")
    key = (NCH, phases)
    if key not in _CACHE:
        _CACHE[key] = _build(NCH, phases)
    nc = _CACHE[key]
    in_maps = []
    for r in range(NCORES):
        m = dict(w)
        m.update(per_core[r])
        in_maps.append(m)
    import os
    trace = bool(os.environ.get("KERNEL_TRACE"))
    if trace:
        try:
            import axon_prof
            axon_prof.install()
        except Exception:
            trace = False
    res = run_bass_kernel_spmd(nc, in_maps, core_ids=list(range(NCORES)), trace=trace)
    if trace and res.exec_time_ns is not None:
        print(f"HW exec time: {res.exec_time_ns} ns")
        if res.per_core_scope_times:
            for scope, cores in sorted(res.per_core_scope_times.items()):
                print(f"  scope {scope}: {cores}")
    global _LAST_RES
    _LAST_RES = res
    out = res.results[0]["out"]
    return out.reshape(G, 2, 3).astype(np.float32)
